# revision 2
# baseline (speedup 1.0000x reference)
"""GCNII (8-layer) graph-conv kernel for 8 TRN2 NeuronCores.

Node sharding: core c owns a contiguous block of destination nodes and all
edges into them. Per layer:
  gather   - dma_gather fetches each edge's source-feature row (256B) from a
             per-core DRAM replica of the full node table (bf16, feature dim
             padded 64->128 so rows are 256B = dma_gather granularity).
  scatter  - edges are grouped by 128-node dst windows; for each 128-edge
             tile one DVE tensor_scalar builds S[p,d] = (iota==dst_p)*w_p
             and the tensor engine accumulates msg^T @ S into a PSUM tile
             [64 feat x 128 dst].
  update   - support = psum + 0.1*x0 (the 0.9 edge scale is folded into the
             edge weights); h = relu(support @ Weff[l]) with
             Weff = (1-beta)I + beta*conv_W folded on the host.
  exchange - AllGather of each core's new h block rebuilds the table.

All cores run one SPMD program, so bucket sizes are equalized across cores on
the host (max over cores, rounded up to 128-edge tiles).
"""

import os
import sys

import numpy as np

for _p in ("/opt/trn_rl_repo", os.path.expanduser("~/trn_rl_repo")):
    if os.path.isdir(_p) and _p not in sys.path:
        sys.path.insert(0, _p)

import ml_dtypes  # noqa: E402

import concourse.bacc as bacc  # noqa: E402
import concourse.bass as bass  # noqa: E402
import concourse.mybir as mybir  # noqa: E402
import concourse.tile as tile  # noqa: E402

BF16 = mybir.dt.bfloat16
F32 = mybir.dt.float32
F32R = mybir.dt.float32r
I16 = mybir.dt.int16
NP_BF16 = ml_dtypes.bfloat16
AF = mybir.ActivationFunctionType
ALU = mybir.AluOpType


class Cfg:
    def __init__(
        self,
        n_nodes=100000,
        f_in=256,
        n_edges=3200000,
        n_layers=8,
        alpha=0.1,
        theta=0.5,
        n_cores=8,
        n_sb=4,
        group_w=2,
        use_for_i=False,  # collectives break inside Tile For_i loops
    ):
        self.n_nodes = n_nodes
        self.f_in = f_in
        self.h = 64
        self.n_edges = n_edges
        self.n_layers = n_layers
        self.alpha = alpha
        self.theta = theta
        self.n_cores = n_cores
        self.n_sb = n_sb
        self.group_w = group_w
        self.use_for_i = use_for_i

        assert n_nodes % n_cores == 0
        self.n_loc = n_nodes // n_cores
        self.n_win = (self.n_loc + 127) // 128
        self.n_loc_pad = self.n_win * 128
        self.n_tab = self.n_loc_pad * n_cores
        assert self.n_tab % n_sb == 0
        self.sb_rows = self.n_tab // n_sb
        assert self.sb_rows <= 32767, "int16 gather index limit"
        self.n_grp = (self.n_win + group_w - 1) // group_w
        assert f_in % 128 == 0

    def wins(self, g):
        return range(g * self.group_w, min((g + 1) * self.group_w, self.n_win))


# ---------------------------------------------------------------------------
# host preprocessing
# ---------------------------------------------------------------------------


def preprocess(cfg, x, edge_index, edge_weight, W0, b0, W1, b1, conv_W):
    C, H = cfg.n_cores, cfg.h
    src = np.asarray(edge_index[0], dtype=np.int64)
    dst = np.asarray(edge_index[1], dtype=np.int64)
    wgt = np.asarray(edge_weight, dtype=np.float32) * (1.0 - cfg.alpha)

    core = dst // cfg.n_loc
    loc = dst - core * cfg.n_loc
    win = loc // 128
    dl = (loc - win * 128).astype(np.float32)
    prow = src // cfg.n_loc * cfg.n_loc_pad + src % cfg.n_loc
    sb = prow // cfg.sb_rows
    rel = (prow - sb * cfg.sb_rows).astype(np.int16)

    key = (core * cfg.n_sb + sb) * cfg.n_win + win
    cnt = np.bincount(key, minlength=C * cfg.n_sb * cfg.n_win).reshape(
        C, cfg.n_sb, cfg.n_win
    )
    pad = ((cnt.max(axis=0) + 127) // 128) * 128  # [n_sb, n_win]
    ntile = pad // 128

    # stream layout: [g][sb][w in group] padded buckets
    bucket_off = np.zeros((cfg.n_sb, cfg.n_win), dtype=np.int64)
    chunk_base = np.zeros((cfg.n_grp, cfg.n_sb), dtype=np.int64)
    n_gsb = np.zeros((cfg.n_grp, cfg.n_sb), dtype=np.int64)
    idx_off = np.zeros((cfg.n_grp, cfg.n_sb), dtype=np.int64)
    off = 0
    for g in range(cfg.n_grp):
        for s in range(cfg.n_sb):
            chunk_base[g, s] = off
            idx_off[g, s] = off // 16
            for w in cfg.wins(g):
                bucket_off[s, w] = off
                off += int(pad[s, w])
            n_gsb[g, s] = off - chunk_base[g, s]
    tot_slots = off
    t_tot = tot_slots // 128

    # S-column order [g][w][sb][tile] -> permutation of stream tiles
    perm = []
    grp_t0 = np.zeros(cfg.n_grp + 1, dtype=np.int64)
    for g in range(cfg.n_grp):
        grp_t0[g] = len(perm)
        for w in cfg.wins(g):
            for s in range(cfg.n_sb):
                t0 = bucket_off[s, w] // 128
                perm.extend(range(t0, t0 + int(ntile[s, w])))
    grp_t0[cfg.n_grp] = len(perm)
    perm = np.asarray(perm, dtype=np.int64)
    assert perm.size == t_tot

    order = np.lexsort((win, sb, core))
    beta = np.log(cfg.theta / (np.arange(cfg.n_layers) + 1) + 1.0).astype(
        np.float32
    )
    weff = np.zeros((H, cfg.n_layers * H), dtype=np.float32)
    for l in range(cfg.n_layers):
        weff[:, l * H : (l + 1) * H] = (1.0 - beta[l]) * np.eye(
            H, dtype=np.float32
        ) + beta[l] * np.asarray(conv_W[l], dtype=np.float32)

    x = np.asarray(x, dtype=np.float32)
    consts = {
        "w0": np.asarray(W0, dtype=np.float32),
        "b0": np.asarray(b0, dtype=np.float32).reshape(H, 1).copy(),
        "w1": np.asarray(W1, dtype=np.float32).astype(NP_BF16),
        "b1": np.asarray(b1, dtype=np.float32).reshape(H, 1).copy(),
        "weff": weff.astype(NP_BF16),
        "iota": np.tile(np.arange(128, dtype=np.float32), (128, 1)).astype(
            NP_BF16
        ),
        "identb": np.eye(128, dtype=np.float32).astype(NP_BF16),
        "identf": np.eye(128, dtype=np.float32),
    }

    in_maps = []
    for c in range(C):
        m = core[order] == c
        o = order[m]
        sb_c, win_c = sb[o], win[o]
        bkey = sb_c * cfg.n_win + win_c
        bc = np.bincount(bkey, minlength=cfg.n_sb * cfg.n_win)
        first = np.zeros(cfg.n_sb * cfg.n_win, dtype=np.int64)
        first[1:] = np.cumsum(bc)[:-1]
        rank = np.arange(bkey.size) - first[bkey]
        slot = bucket_off[sb_c, win_c] + rank

        idx_full = np.zeros(tot_slots, dtype=np.int16)
        dl_full = np.zeros(tot_slots, dtype=np.float32)
        w_full = np.zeros(tot_slots, dtype=np.float32)
        idx_full[slot] = rel[o]
        dl_full[slot] = dl[o]
        w_full[slot] = wgt[o]

        # gather index layout: [16, n/16] per region, tiled to 128 partitions
        idx_arr = np.tile(
            idx_full.reshape(tot_slots // 16, 16).T, (8, 1)
        )  # [128, tot_slots/16]

        dl_mat = dl_full.reshape(t_tot, 128).T
        w_mat = w_full.reshape(t_tot, 128).T
        dl_arr = np.ascontiguousarray(dl_mat[:, perm])
        w_arr = np.ascontiguousarray(w_mat[:, perm])

        xT = np.zeros((cfg.f_in, cfg.n_loc_pad), dtype=np.float32)
        xT[:, : cfg.n_loc] = x[c * cfg.n_loc : (c + 1) * cfg.n_loc].T

        in_maps.append(
            dict(consts, xT=xT, idx=idx_arr, dstc=dl_arr, wc=w_arr)
        )

    meta = dict(
        pad=pad,
        ntile=ntile,
        n_gsb=n_gsb,
        bucket_off=bucket_off,
        chunk_base=chunk_base,
        idx_off=idx_off,
        grp_t0=grp_t0,
        tot_slots=tot_slots,
        t_tot=t_tot,
    )
    return in_maps, meta


# ---------------------------------------------------------------------------
# device program
# ---------------------------------------------------------------------------


def build_program(cfg, meta):
    import contextlib

    H = cfg.h
    NW = cfg.n_win
    NSB = cfg.n_sb
    NLP = cfg.n_loc_pad
    ntile, n_gsb = meta["ntile"], meta["n_gsb"]
    kf = cfg.f_in // 128
    nchunk = (NLP + 511) // 512

    nc = bacc.Bacc("TRN2", target_bir_lowering=False)

    xT_d = nc.declare_dram_parameter("xT", [cfg.f_in, NLP], F32, isOutput=False)
    idx_d = nc.declare_dram_parameter(
        "idx", [128, meta["tot_slots"] // 16], I16, isOutput=False
    )
    dstc_d = nc.declare_dram_parameter(
        "dstc", [128, meta["t_tot"]], F32, isOutput=False
    )
    wc_d = nc.declare_dram_parameter("wc", [128, meta["t_tot"]], F32, isOutput=False)
    w0_d = nc.declare_dram_parameter("w0", [cfg.f_in, H], F32, isOutput=False)
    b0_d = nc.declare_dram_parameter("b0", [H, 1], F32, isOutput=False)
    w1_d = nc.declare_dram_parameter("w1", [H, H], BF16, isOutput=False)
    b1_d = nc.declare_dram_parameter("b1", [H, 1], F32, isOutput=False)
    weff_d = nc.declare_dram_parameter(
        "weff", [H, cfg.n_layers * H], BF16, isOutput=False
    )
    iota_d = nc.declare_dram_parameter("iota", [128, 128], BF16, isOutput=False)
    identb_d = nc.declare_dram_parameter("identb", [128, 128], BF16, isOutput=False)
    identf_d = nc.declare_dram_parameter("identf", [128, 128], F32, isOutput=False)
    out_d = nc.declare_dram_parameter("out", [NLP, H], F32, isOutput=True)

    from concourse import library_config

    with tile.TileContext(nc) as tc, contextlib.ExitStack() as ctx:
        nc.gpsimd.load_library(library_config.mlp)
        dram = ctx.enter_context(tc.tile_pool(name="dram", bufs=1, space="DRAM"))
        consts = ctx.enter_context(tc.tile_pool(name="consts", bufs=1))
        big = ctx.enter_context(tc.tile_pool(name="big", bufs=1))
        xs_pool = ctx.enter_context(tc.tile_pool(name="xs", bufs=3))
        idx_pool = ctx.enter_context(tc.tile_pool(name="idxp", bufs=2))
        dw_pool = ctx.enter_context(tc.tile_pool(name="dwp", bufs=2))
        msg_pools = [
            ctx.enter_context(tc.tile_pool(name=f"msg{s}", bufs=2))
            for s in range(NSB)
        ]
        s_pool = ctx.enter_context(tc.tile_pool(name="sp", bufs=6))
        stage_pool = ctx.enter_context(tc.tile_pool(name="stg", bufs=1))
        ps_agg = ctx.enter_context(tc.tile_pool(name="ps_agg", bufs=4, space="PSUM"))
        ps_big = ctx.enter_context(tc.tile_pool(name="ps_big", bufs=2, space="PSUM"))
        ps_tr = ctx.enter_context(tc.tile_pool(name="ps_tr", bufs=2, space="PSUM"))

        tables = [
            dram.tile(
                [cfg.n_tab, 128], BF16, addr_space="Shared", name=f"table{i}"
            )
            for i in range(cfg.n_layers + 1)
        ]
        staging = dram.tile([NLP, 128], BF16)

        iota_t = consts.tile([128, 128], BF16)
        identb_t = consts.tile([128, 128], BF16)
        identf_t = consts.tile([128, 128], F32)
        w0_t = consts.tile([128, kf * H], F32)
        w1_t = consts.tile([H, H], BF16)
        b0_t = consts.tile([H, 1], F32)
        b1_t = consts.tile([H, 1], F32)
        weff_t = consts.tile([H, cfg.n_layers * H], BF16)

        nc.sync.dma_start(iota_t[:], iota_d[:])
        nc.sync.dma_start(identb_t[:], identb_d[:])
        nc.sync.dma_start(identf_t[:], identf_d[:])
        for k in range(kf):
            nc.sync.dma_start(
                w0_t[:, k * H : (k + 1) * H], w0_d[k * 128 : (k + 1) * 128, :]
            )
        nc.sync.dma_start(w1_t[:], w1_d[:])
        nc.sync.dma_start(b0_t[:], b0_d[:])
        nc.sync.dma_start(b1_t[:], b1_d[:])
        nc.sync.dma_start(weff_t[:], weff_d[:])

        x0s = big.tile([H, NLP], BF16)
        support = big.tile([H, NLP], BF16)
        hT = big.tile([H, NLP], BF16)
        stage_s = stage_pool.tile([128, NW * H], BF16)

        # zero the feature-pad half of the staging buffer, once (reuse
        # stage_s as the zero source; it is rewritten by every layer anyway)
        nc.vector.memset(stage_s[:], 0)
        nc.sync.dma_start(
            staging[:].rearrange("(w p) f -> p w f", p=128)[:, :, H:],
            stage_s[:].rearrange("p (w f) -> p w f", f=H),
        )

        # ---------------- prologue: h0 = relu(x @ W0 + b0) -----------------
        for cix in range(nchunk):
            c0 = cix * 512
            cw = min(512, NLP - c0)
            ps = ps_big.tile([H, 512], F32)
            for k in range(kf):
                xt = xs_pool.tile([128, 512], F32)
                nc.sync.dma_start(
                    xt[:, :cw], xT_d[k * 128 : (k + 1) * 128, c0 : c0 + cw]
                )
                nc.tensor.matmul(
                    ps[:, :cw],
                    lhsT=w0_t[:, k * H : (k + 1) * H],
                    rhs=xt[:, :cw],
                    start=(k == 0),
                    stop=(k == kf - 1),
                )
            nc.scalar.activation(
                hT[:, c0 : c0 + cw], ps[:, :cw], AF.Relu, bias=b0_t[:]
            )
            nc.vector.tensor_scalar_mul(
                x0s[:, c0 : c0 + cw], hT[:, c0 : c0 + cw], cfg.alpha
            )

        def stage_and_allgather(table):
            for w in range(NW):
                pst = ps_tr.tile([128, H], BF16)
                nc.tensor.transpose(
                    out=pst[:],
                    in_=hT[:, w * 128 : (w + 1) * 128],
                    identity=identb_t[:H, :H],
                )
                nc.vector.tensor_copy(
                    out=stage_s[:, w * H : (w + 1) * H], in_=pst[:]
                )
            nc.sync.dma_start(
                staging[:].rearrange("(w p) f -> p w f", p=128)[:, :, :H],
                stage_s[:].rearrange("p (w f) -> p w f", f=H),
            )
            nc.gpsimd.collective_compute(
                "AllGather",
                ALU.bypass,
                ins=[staging[:].opt()],
                outs=[table[:].opt()],
                replica_groups=[list(range(cfg.n_cores))],
            )

        stage_and_allgather(tables[0])

        # ---------------- per-layer body -----------------------------------
        def layer_body(li):
            table = tables[li]
            nc.vector.tensor_copy(out=support[:], in_=x0s[:])
            tcol = 0
            for g in range(cfg.n_grp):
                tg = int(meta["grp_t0"][g + 1] - meta["grp_t0"][g])
                t0g = int(meta["grp_t0"][g])
                idxn = int(n_gsb[g].sum()) // 16
                if idxn:
                    idx_t = idx_pool.tile([128, idxn], I16)
                    nc.sync.dma_start(
                        idx_t[:],
                        idx_d[:, int(meta["idx_off"][g, 0]) :][:, :idxn],
                    )
                if tg:
                    dst_t = dw_pool.tile([128, tg], F32, tag="dst")
                    w_t = dw_pool.tile([128, tg], F32, tag="w")
                    nc.sync.dma_start(dst_t[:], dstc_d[:, t0g : t0g + tg])
                    nc.sync.dma_start(w_t[:], wc_d[:, t0g : t0g + tg])

                msgs = {}
                for s in range(NSB):
                    n = int(n_gsb[g, s])
                    if n == 0:
                        continue
                    mt = msg_pools[s].tile([128, n // 128, 128], BF16)
                    ic = int(meta["idx_off"][g, s] - meta["idx_off"][g, 0])
                    nc.gpsimd.dma_gather(
                        mt[:],
                        table[s * cfg.sb_rows : (s + 1) * cfg.sb_rows, :],
                        idx_t[:, ic : ic + n // 16],
                        n,
                        n,
                        128,
                        single_packet=False,  # single-packet mode breaks >1k idxs
                    )
                    msgs[s] = mt

                for w in cfg.wins(g):
                    kt = int(sum(ntile[s, w] for s in range(NSB)))
                    if kt == 0:
                        continue
                    ps = ps_agg.tile([H, 128], F32)
                    k = 0
                    for s in range(NSB):
                        nt = int(ntile[s, w])
                        if nt == 0:
                            continue
                        i0 = int(
                            meta["bucket_off"][s, w] - meta["chunk_base"][g, s]
                        ) // 128
                        for i in range(nt):
                            st = s_pool.tile([128, 128], BF16)
                            nc.vector.tensor_scalar(
                                out=st[:],
                                in0=iota_t[:],
                                scalar1=dst_t[:, tcol - t0g : tcol - t0g + 1],
                                scalar2=w_t[:, tcol - t0g : tcol - t0g + 1],
                                op0=ALU.is_equal,
                                op1=ALU.mult,
                            )
                            nc.tensor.matmul(
                                ps[:],
                                lhsT=msgs[s][:, i0 + i, :H],
                                rhs=st[:],
                                start=(k == 0),
                                stop=(k == kt - 1),
                            )
                            k += 1
                            tcol += 1
                    nc.vector.tensor_tensor(
                        out=support[:, w * 128 : (w + 1) * 128],
                        in0=support[:, w * 128 : (w + 1) * 128],
                        in1=ps[:],
                        op=ALU.add,
                    )

            if isinstance(li, int):
                wsl = weff_t[:, li * H : (li + 1) * H]
            else:
                # walrus can't take a register offset in ldweights: copy the
                # dynamic Weff slice to a fixed location first
                wsl_t = consts.tile([H, H], BF16, tag="wsl")
                nc.vector.tensor_copy(
                    out=wsl_t[:], in_=weff_t[:, bass.ds(li * H, H)]
                )
                wsl = wsl_t[:]
            for cix in range(nchunk):
                c0 = cix * 512
                cw = min(512, NLP - c0)
                ps = ps_big.tile([H, 512], F32)
                nc.tensor.matmul(
                    ps[:, :cw],
                    lhsT=wsl,
                    rhs=support[:, c0 : c0 + cw],
                    start=True,
                    stop=True,
                )
                nc.scalar.activation(hT[:, c0 : c0 + cw], ps[:, :cw], AF.Relu)
            stage_and_allgather(tables[li + 1])

        if cfg.use_for_i:
            with tc.For_i(0, cfg.n_layers, 1) as li:
                layer_body(li)
        else:
            for li in range(cfg.n_layers):
                layer_body(li)

        # ---------------- epilogue: logits + log_softmax, chunk-wise --------
        out_view = out_d[:].rearrange("(w p) f -> p w f", p=128)
        for cix in range(nchunk):
            c0 = cix * 512
            cw = min(512, NLP - c0)
            nwc = cw // 128  # windows in this chunk
            ps = ps_big.tile([H, 512], F32)
            nc.tensor.matmul(
                ps[:, :cw], lhsT=w1_t[:], rhs=hT[:, c0 : c0 + cw],
                start=True, stop=True,
            )
            lt = xs_pool.tile([H, 512], F32, tag="lt")
            nc.scalar.activation(
                lt[:, :cw], ps[:, :cw], AF.Identity, bias=b1_t[:]
            )
            xch = xs_pool.tile([128, nwc * H], F32, tag="xch")
            for wi in range(nwc):
                pst = ps_tr.tile([128, H], F32)
                nc.tensor.transpose(
                    out=pst[:],
                    in_=lt[:, wi * 128 : (wi + 1) * 128],
                    identity=identf_t[:H, :H],
                )
                nc.vector.tensor_copy(
                    out=xch[:, wi * H : (wi + 1) * H], in_=pst[:]
                )
            xch3 = xch[:].rearrange("p (w f) -> p w f", f=H)
            mx = xs_pool.tile([128, nwc], F32, tag="mx")
            nc.vector.tensor_reduce(
                out=mx[:], in_=xch3, axis=mybir.AxisListType.X, op=ALU.max
            )
            nc.vector.tensor_tensor(
                out=xch3,
                in0=xch3,
                in1=mx[:].unsqueeze(2).to_broadcast([128, nwc, H]),
                op=ALU.subtract,
            )
            enm = xs_pool.tile([128, nwc * H], F32, tag="enm")
            nc.scalar.activation(enm[:], xch[:], AF.Exp)
            ssum = xs_pool.tile([128, nwc], F32, tag="ssum")
            nc.vector.tensor_reduce(
                out=ssum[:],
                in_=enm[:].rearrange("p (w f) -> p w f", f=H),
                axis=mybir.AxisListType.X,
                op=ALU.add,
            )
            lsum = xs_pool.tile([128, nwc], F32, tag="lsum")
            nc.scalar.activation(lsum[:], ssum[:], AF.Ln)
            nc.vector.tensor_tensor(
                out=xch3,
                in0=xch3,
                in1=lsum[:].unsqueeze(2).to_broadcast([128, nwc, H]),
                op=ALU.subtract,
            )
            nc.sync.dma_start(
                out_view[:, cix * 4 : cix * 4 + nwc, :], xch3
            )

    nc.compile()
    return nc


# ---------------------------------------------------------------------------
# entry point
# ---------------------------------------------------------------------------


LAST_EXEC_NS = None
LAST_TRACE = None


def kernel(x, edge_index, edge_weight, W0, b0, W1, b1, conv_W):
    global LAST_EXEC_NS, LAST_TRACE
    from concourse.bass_utils import run_bass_kernel_spmd

    cfg = Cfg()
    in_maps, meta = preprocess(
        cfg, x, edge_index, edge_weight, W0, b0, W1, b1, conv_W
    )
    nc = build_program(cfg, meta)
    res = run_bass_kernel_spmd(nc, in_maps, list(range(cfg.n_cores)))
    LAST_EXEC_NS = getattr(res, "exec_time_ns", None)
    LAST_TRACE = getattr(res, "instructions_and_trace", None)
    outs = res.results
    full = np.concatenate(
        [np.asarray(outs[c]["out"])[: cfg.n_loc] for c in range(cfg.n_cores)],
        axis=0,
    )
    return full.astype(np.float32)



# revision 16
# speedup vs baseline: 2.3552x; 2.3552x over previous
"""GCNII (8-layer) graph-conv kernel for 8 TRN2 NeuronCores.

Node sharding: core c owns a contiguous block of destination nodes and all
edges into them. Per layer:
  gather   - dma_gather fetches each edge's source-feature row (256B) from a
             per-core DRAM replica of the full node table (bf16, feature dim
             padded 64->128 so rows are 256B = dma_gather granularity).
  scatter  - edges are grouped by 128-node dst windows; for each 128-edge
             tile one DVE tensor_scalar builds S[p,d] = (iota==dst_p)*w_p
             and the tensor engine accumulates msg^T @ S into a PSUM tile
             [64 feat x 128 dst].
  update   - support = psum + 0.1*x0 (the 0.9 edge scale is folded into the
             edge weights); h = relu(support @ Weff[l]) with
             Weff = (1-beta)I + beta*conv_W folded on the host.
  exchange - AllGather of each core's new h block rebuilds the table.

All cores run one SPMD program, so bucket sizes are equalized across cores on
the host (max over cores, rounded up to 128-edge tiles).
"""

import os
import sys

import numpy as np

for _p in ("/opt/trn_rl_repo", os.path.expanduser("~/trn_rl_repo")):
    if os.path.isdir(_p) and _p not in sys.path:
        sys.path.insert(0, _p)

import ml_dtypes  # noqa: E402

import concourse.bacc as bacc  # noqa: E402
import concourse.bass as bass  # noqa: E402
import concourse.mybir as mybir  # noqa: E402
import concourse.tile as tile  # noqa: E402

BF16 = mybir.dt.bfloat16
F32 = mybir.dt.float32
F32R = mybir.dt.float32r
I16 = mybir.dt.int16
NP_BF16 = ml_dtypes.bfloat16
AF = mybir.ActivationFunctionType
ALU = mybir.AluOpType


class Cfg:
    def __init__(
        self,
        n_nodes=100000,
        f_in=256,
        n_edges=3200000,
        n_layers=8,
        alpha=0.1,
        theta=0.5,
        n_cores=8,
        n_sb=4,
        group_w=2,
        use_for_i=False,  # collectives break inside Tile For_i loops
    ):
        self.n_nodes = n_nodes
        self.f_in = f_in
        self.h = 64
        self.n_edges = n_edges
        self.n_layers = n_layers
        self.alpha = alpha
        self.theta = theta
        self.n_cores = n_cores
        self.n_sb = n_sb
        self.group_w = group_w
        self.use_for_i = use_for_i

        assert n_nodes % n_cores == 0
        self.n_loc = n_nodes // n_cores
        self.n_win = (self.n_loc + 127) // 128
        self.n_loc_pad = self.n_win * 128
        self.n_tab = self.n_loc_pad * n_cores
        assert self.n_tab % n_sb == 0
        self.sb_rows = self.n_tab // n_sb
        assert self.sb_rows <= 32767, "int16 gather index limit"
        self.n_grp = (self.n_win + group_w - 1) // group_w
        assert f_in % 128 == 0

    def wins(self, g):
        return range(g * self.group_w, min((g + 1) * self.group_w, self.n_win))


# ---------------------------------------------------------------------------
# host preprocessing
# ---------------------------------------------------------------------------


def preprocess(cfg, x, edge_index, edge_weight, W0, b0, W1, b1, conv_W):
    C, H = cfg.n_cores, cfg.h
    src = np.asarray(edge_index[0], dtype=np.int64)
    dst = np.asarray(edge_index[1], dtype=np.int64)
    wgt = np.asarray(edge_weight, dtype=np.float32) * (1.0 - cfg.alpha)

    core = dst // cfg.n_loc
    loc = dst - core * cfg.n_loc
    win = loc // 128
    dl = (loc - win * 128).astype(NP_BF16)
    prow = src // cfg.n_loc * cfg.n_loc_pad + src % cfg.n_loc
    sb = prow // cfg.sb_rows
    rel = (prow - sb * cfg.sb_rows).astype(np.int16)

    key = (core * cfg.n_sb + sb) * cfg.n_win + win
    cnt = np.bincount(key, minlength=C * cfg.n_sb * cfg.n_win).reshape(
        C, cfg.n_sb, cfg.n_win
    )
    pad = ((cnt.max(axis=0) + 127) // 128) * 128  # [n_sb, n_win]
    ntile = pad // 128

    # stream layout: [g][sb][w in group] padded buckets
    bucket_off = np.zeros((cfg.n_sb, cfg.n_win), dtype=np.int64)
    chunk_base = np.zeros((cfg.n_grp, cfg.n_sb), dtype=np.int64)
    n_gsb = np.zeros((cfg.n_grp, cfg.n_sb), dtype=np.int64)
    idx_off = np.zeros((cfg.n_grp, cfg.n_sb), dtype=np.int64)
    off = 0
    for g in range(cfg.n_grp):
        for s in range(cfg.n_sb):
            chunk_base[g, s] = off
            idx_off[g, s] = off // 16
            for w in cfg.wins(g):
                bucket_off[s, w] = off
                off += int(pad[s, w])
            n_gsb[g, s] = off - chunk_base[g, s]
    tot_slots = off
    t_tot = tot_slots // 128

    order = np.lexsort((win, sb, core))
    beta = np.log(cfg.theta / (np.arange(cfg.n_layers) + 1) + 1.0).astype(
        np.float32
    )
    weff = np.zeros((H, cfg.n_layers * H), dtype=np.float32)
    for l in range(cfg.n_layers):
        weff[:, l * H : (l + 1) * H] = (1.0 - beta[l]) * np.eye(
            H, dtype=np.float32
        ) + beta[l] * np.asarray(conv_W[l], dtype=np.float32)

    x = np.asarray(x, dtype=np.float32)
    ntmax = int(n_gsb.max()) // 128
    consts = {
        "iotaw": np.tile(
            np.tile(np.arange(128, dtype=np.float32), ntmax), (128, 1)
        ).astype(NP_BF16),
        "w0": np.asarray(W0, dtype=np.float32),
        "b0": np.asarray(b0, dtype=np.float32).reshape(H, 1).copy(),
        "w1": np.asarray(W1, dtype=np.float32).astype(NP_BF16),
        "b1": np.asarray(b1, dtype=np.float32).reshape(H, 1).copy(),
        "weff": weff.astype(NP_BF16),
        "iota": np.tile(np.arange(128, dtype=np.float32), (128, 1)).astype(
            NP_BF16
        ),
        "identb": np.eye(128, dtype=np.float32).astype(NP_BF16),
        "identf": np.eye(128, dtype=np.float32),
    }

    in_maps = []
    for c in range(C):
        m = core[order] == c
        o = order[m]
        sb_c, win_c = sb[o], win[o]
        bkey = sb_c * cfg.n_win + win_c
        bc = np.bincount(bkey, minlength=cfg.n_sb * cfg.n_win)
        first = np.zeros(cfg.n_sb * cfg.n_win, dtype=np.int64)
        first[1:] = np.cumsum(bc)[:-1]
        rank = np.arange(bkey.size) - first[bkey]
        slot = bucket_off[sb_c, win_c] + rank

        idx_full = np.zeros(tot_slots, dtype=np.int16)
        dl_full = np.zeros(tot_slots, dtype=NP_BF16)
        w_full = np.zeros(tot_slots, dtype=NP_BF16)
        idx_full[slot] = rel[o]
        dl_full[slot] = dl[o]
        w_full[slot] = wgt[o].astype(NP_BF16)

        # gather index layout: [16, n/16] per region, tiled to 128 partitions
        idx_arr = np.tile(
            idx_full.reshape(tot_slots // 16, 16).T, (8, 1)
        )  # [128, tot_slots/16]

        # stream-tile order (no perm): column t = stream tile t
        dl_arr = np.ascontiguousarray(dl_full.reshape(t_tot, 128).T)
        w_arr = np.ascontiguousarray(w_full.reshape(t_tot, 128).T)

        xT = np.zeros((cfg.f_in, cfg.n_loc_pad), dtype=np.float32)
        xT[:, : cfg.n_loc] = x[c * cfg.n_loc : (c + 1) * cfg.n_loc].T

        in_maps.append(
            dict(consts, xT=xT, idx=idx_arr, dstc=dl_arr, wc=w_arr)
        )

    meta = dict(
        pad=pad,
        ntile=ntile,
        n_gsb=n_gsb,
        bucket_off=bucket_off,
        chunk_base=chunk_base,
        idx_off=idx_off,
        tot_slots=tot_slots,
        t_tot=t_tot,
        ntmax=ntmax,
    )
    return in_maps, meta


# ---------------------------------------------------------------------------
# device program
# ---------------------------------------------------------------------------


def build_program(cfg, meta):
    import contextlib

    H = cfg.h
    NW = cfg.n_win
    NSB = cfg.n_sb
    NLP = cfg.n_loc_pad
    ntile, n_gsb = meta["ntile"], meta["n_gsb"]
    kf = cfg.f_in // 128
    nchunk = (NLP + 511) // 512

    nc = bacc.Bacc("TRN2", target_bir_lowering=False, num_swdge_queues=4)

    xT_d = nc.declare_dram_parameter("xT", [cfg.f_in, NLP], F32, isOutput=False)
    idx_d = nc.declare_dram_parameter(
        "idx", [128, meta["tot_slots"] // 16], I16, isOutput=False
    )
    dstc_d = nc.declare_dram_parameter(
        "dstc", [128, meta["t_tot"]], BF16, isOutput=False
    )
    wc_d = nc.declare_dram_parameter("wc", [128, meta["t_tot"]], BF16, isOutput=False)
    w0_d = nc.declare_dram_parameter("w0", [cfg.f_in, H], F32, isOutput=False)
    b0_d = nc.declare_dram_parameter("b0", [H, 1], F32, isOutput=False)
    w1_d = nc.declare_dram_parameter("w1", [H, H], BF16, isOutput=False)
    b1_d = nc.declare_dram_parameter("b1", [H, 1], F32, isOutput=False)
    weff_d = nc.declare_dram_parameter(
        "weff", [H, cfg.n_layers * H], BF16, isOutput=False
    )
    iota_d = nc.declare_dram_parameter("iota", [128, 128], BF16, isOutput=False)
    iotaw_d = nc.declare_dram_parameter(
        "iotaw", [128, meta["ntmax"] * 128], BF16, isOutput=False
    )
    identb_d = nc.declare_dram_parameter("identb", [128, 128], BF16, isOutput=False)
    identf_d = nc.declare_dram_parameter("identf", [128, 128], F32, isOutput=False)
    out_d = nc.declare_dram_parameter("out", [NLP, H], F32, isOutput=True)

    from concourse import library_config

    with tile.TileContext(nc) as tc, contextlib.ExitStack() as ctx:
        nc.gpsimd.load_library(library_config.mlp)
        dram = ctx.enter_context(tc.tile_pool(name="dram", bufs=1, space="DRAM"))
        consts = ctx.enter_context(tc.tile_pool(name="consts", bufs=1))
        big = ctx.enter_context(tc.tile_pool(name="big", bufs=1))
        xs_pool = ctx.enter_context(tc.tile_pool(name="xs", bufs=3))
        idx_pool = ctx.enter_context(tc.tile_pool(name="idxp", bufs=2))
        dw_pool = ctx.enter_context(tc.tile_pool(name="dwp", bufs=2))
        msg_pools = [
            ctx.enter_context(tc.tile_pool(name=f"msg{s}", bufs=2))
            for s in range(NSB)
        ]
        sch_pools = [
            ctx.enter_context(tc.tile_pool(name=f"sch{s}", bufs=2))
            for s in range(NSB)
        ]
        tmp_pool = ctx.enter_context(tc.tile_pool(name="schtmp", bufs=2))
        stage_pool = ctx.enter_context(tc.tile_pool(name="stg", bufs=1))
        ps_agg = ctx.enter_context(tc.tile_pool(name="ps_agg", bufs=4, space="PSUM"))
        ps_big = ctx.enter_context(tc.tile_pool(name="ps_big", bufs=2, space="PSUM"))
        ps_tr = ctx.enter_context(tc.tile_pool(name="ps_tr", bufs=2, space="PSUM"))

        tables = [
            dram.tile(
                [cfg.n_tab, 128], BF16, addr_space="Shared", name=f"table{i}"
            )
            for i in range(cfg.n_layers + 1)
        ]
        staging = dram.tile([NLP, 128], BF16)

        iota_t = consts.tile([128, 128], BF16)
        iotaw_t = consts.tile([128, meta["ntmax"] * 128], BF16)
        identb_t = consts.tile([128, 128], BF16)
        identf_t = consts.tile([128, 128], F32)
        w0_t = consts.tile([128, kf * H], F32)
        w1_t = consts.tile([H, H], BF16)
        b0_t = consts.tile([H, 1], F32)
        b1_t = consts.tile([H, 1], F32)
        weff_t = consts.tile([H, cfg.n_layers * H], BF16)

        nc.sync.dma_start(iota_t[:], iota_d[:])
        nc.sync.dma_start(iotaw_t[:], iotaw_d[:])
        nc.sync.dma_start(identb_t[:], identb_d[:])
        nc.sync.dma_start(identf_t[:], identf_d[:])
        for k in range(kf):
            nc.sync.dma_start(
                w0_t[:, k * H : (k + 1) * H], w0_d[k * 128 : (k + 1) * 128, :]
            )
        nc.sync.dma_start(w1_t[:], w1_d[:])
        nc.sync.dma_start(b0_t[:], b0_d[:])
        nc.sync.dma_start(b1_t[:], b1_d[:])
        nc.sync.dma_start(weff_t[:], weff_d[:])

        x0s = big.tile([H, NLP], BF16)
        support = big.tile([H, NLP], BF16)
        hT = big.tile([H, NLP], BF16)
        stage_s = stage_pool.tile([128, NW * H], BF16)

        # zero the feature-pad half of the staging buffer, once (reuse
        # stage_s as the zero source; it is rewritten by every layer anyway)
        nc.vector.memset(stage_s[:], 0)
        nc.sync.dma_start(
            staging[:].rearrange("(w p) f -> p w f", p=128)[:, :, H:],
            stage_s[:].rearrange("p (w f) -> p w f", f=H),
        )

        # ---------------- prologue: h0 = relu(x @ W0 + b0) -----------------
        for cix in range(nchunk):
            c0 = cix * 512
            cw = min(512, NLP - c0)
            ps = ps_big.tile([H, 512], F32)
            for k in range(kf):
                xt = xs_pool.tile([128, 512], F32)
                nc.sync.dma_start(
                    xt[:, :cw], xT_d[k * 128 : (k + 1) * 128, c0 : c0 + cw]
                )
                nc.tensor.matmul(
                    ps[:, :cw],
                    lhsT=w0_t[:, k * H : (k + 1) * H],
                    rhs=xt[:, :cw],
                    start=(k == 0),
                    stop=(k == kf - 1),
                )
            nc.scalar.activation(
                hT[:, c0 : c0 + cw], ps[:, :cw], AF.Relu, bias=b0_t[:]
            )
            nc.vector.tensor_scalar_mul(
                x0s[:, c0 : c0 + cw], hT[:, c0 : c0 + cw], cfg.alpha
            )

        def stage_and_allgather(table):
            for w in range(NW):
                pst = ps_tr.tile([128, H], BF16)
                nc.tensor.transpose(
                    out=pst[:],
                    in_=hT[:, w * 128 : (w + 1) * 128],
                    identity=identb_t[:H, :H],
                )
                nc.vector.tensor_copy(
                    out=stage_s[:, w * H : (w + 1) * H], in_=pst[:]
                )
            nc.sync.dma_start(
                staging[:].rearrange("(w p) f -> p w f", p=128)[:, :, :H],
                stage_s[:].rearrange("p (w f) -> p w f", f=H),
            )
            nc.gpsimd.collective_compute(
                "AllGather",
                ALU.bypass,
                ins=[staging[:].opt()],
                outs=[table[:].opt()],
                replica_groups=[list(range(cfg.n_cores))],
            )

        stage_and_allgather(tables[0])

        # ---------------- per-layer body -----------------------------------
        def layer_body(li):
            table = tables[li]
            nc.vector.tensor_copy(out=support[:], in_=x0s[:])
            for g in range(cfg.n_grp):
                tg = int(n_gsb[g].sum()) // 128
                t0g = int(meta["chunk_base"][g, 0]) // 128
                idxn = int(n_gsb[g].sum()) // 16
                if idxn:
                    idx_t = idx_pool.tile([128, idxn], I16)
                    nc.sync.dma_start(
                        idx_t[:],
                        idx_d[:, int(meta["idx_off"][g, 0]) :][:, :idxn],
                    )
                if tg:
                    dst_t = dw_pool.tile([128, tg], BF16, tag="dst")
                    w_t = dw_pool.tile([128, tg], BF16, tag="w")
                    nc.sync.dma_start(dst_t[:], dstc_d[:, t0g : t0g + tg])
                    nc.sync.dma_start(w_t[:], wc_d[:, t0g : t0g + tg])

                msgs = {}
                schs = {}
                for s in range(NSB):
                    n = int(n_gsb[g, s])
                    if n == 0:
                        continue
                    nt = n // 128
                    mt = msg_pools[s].tile([128, nt, 128], BF16)
                    ic = int(meta["idx_off"][g, s] - meta["idx_off"][g, 0])
                    nc.gpsimd.dma_gather(
                        mt[:],
                        table[s * cfg.sb_rows : (s + 1) * cfg.sb_rows, :],
                        idx_t[:, ic : ic + n // 16],
                        n,
                        n,
                        128,
                        single_packet=False,  # single-packet mode breaks >1k idxs
                        queue_num=s,
                    )
                    msgs[s] = mt
                    # S chunk for (g, s): S[p, t, d] = (d == dst[p,t]) * w[p,t]
                    # built with tensor_tensor (never takes the 2-port SBUF
                    # lock, so SWDGE descriptor generation is not starved)
                    c0 = int(
                        meta["chunk_base"][g, s] - meta["chunk_base"][g, 0]
                    ) // 128
                    sch = sch_pools[s].tile([128, nt, 128], BF16)
                    tmp = tmp_pool.tile([128, nt, 128], BF16)
                    iow = iotaw_t[:, : nt * 128].rearrange(
                        "p (t d) -> p t d", d=128
                    )
                    nc.vector.tensor_tensor(
                        out=tmp[:],
                        in0=iow,
                        in1=dst_t[:, c0 : c0 + nt]
                        .unsqueeze(2)
                        .to_broadcast([128, nt, 128]),
                        op=ALU.is_equal,
                    )
                    nc.vector.tensor_tensor(
                        out=sch[:],
                        in0=tmp[:],
                        in1=w_t[:, c0 : c0 + nt]
                        .unsqueeze(2)
                        .to_broadcast([128, nt, 128]),
                        op=ALU.mult,
                    )
                    schs[s] = sch

                for w in cfg.wins(g):
                    kt = int(sum(ntile[s, w] for s in range(NSB)))
                    if kt == 0:
                        continue
                    ps = ps_agg.tile([H, 128], F32)
                    k = 0
                    for s in range(NSB):
                        nt = int(ntile[s, w])
                        if nt == 0:
                            continue
                        i0 = int(
                            meta["bucket_off"][s, w] - meta["chunk_base"][g, s]
                        ) // 128
                        for i in range(nt):
                            nc.tensor.matmul(
                                ps[:],
                                lhsT=msgs[s][:, i0 + i, :H],
                                rhs=schs[s][:, i0 + i, :],
                                start=(k == 0),
                                stop=(k == kt - 1),
                            )
                            k += 1
                    nc.vector.tensor_tensor(
                        out=support[:, w * 128 : (w + 1) * 128],
                        in0=support[:, w * 128 : (w + 1) * 128],
                        in1=ps[:],
                        op=ALU.add,
                    )

            if isinstance(li, int):
                wsl = weff_t[:, li * H : (li + 1) * H]
            else:
                # walrus can't take a register offset in ldweights: copy the
                # dynamic Weff slice to a fixed location first
                wsl_t = consts.tile([H, H], BF16, tag="wsl")
                nc.vector.tensor_copy(
                    out=wsl_t[:], in_=weff_t[:, bass.ds(li * H, H)]
                )
                wsl = wsl_t[:]
            for cix in range(nchunk):
                c0 = cix * 512
                cw = min(512, NLP - c0)
                ps = ps_big.tile([H, 512], F32)
                nc.tensor.matmul(
                    ps[:, :cw],
                    lhsT=wsl,
                    rhs=support[:, c0 : c0 + cw],
                    start=True,
                    stop=True,
                )
                nc.scalar.activation(hT[:, c0 : c0 + cw], ps[:, :cw], AF.Relu)
            stage_and_allgather(tables[li + 1])

        if cfg.use_for_i:
            with tc.For_i(0, cfg.n_layers, 1) as li:
                layer_body(li)
        else:
            for li in range(cfg.n_layers):
                layer_body(li)

        # ---------------- epilogue: logits + log_softmax, chunk-wise --------
        out_view = out_d[:].rearrange("(w p) f -> p w f", p=128)
        for cix in range(nchunk):
            c0 = cix * 512
            cw = min(512, NLP - c0)
            nwc = cw // 128  # windows in this chunk
            ps = ps_big.tile([H, 512], F32)
            nc.tensor.matmul(
                ps[:, :cw], lhsT=w1_t[:], rhs=hT[:, c0 : c0 + cw],
                start=True, stop=True,
            )
            lt = xs_pool.tile([H, 512], F32, tag="lt")
            nc.scalar.activation(
                lt[:, :cw], ps[:, :cw], AF.Identity, bias=b1_t[:]
            )
            xch = xs_pool.tile([128, nwc * H], F32, tag="xch")
            for wi in range(nwc):
                pst = ps_tr.tile([128, H], F32)
                nc.tensor.transpose(
                    out=pst[:],
                    in_=lt[:, wi * 128 : (wi + 1) * 128],
                    identity=identf_t[:H, :H],
                )
                nc.vector.tensor_copy(
                    out=xch[:, wi * H : (wi + 1) * H], in_=pst[:]
                )
            xch3 = xch[:].rearrange("p (w f) -> p w f", f=H)
            mx = xs_pool.tile([128, nwc], F32, tag="mx")
            nc.vector.tensor_reduce(
                out=mx[:], in_=xch3, axis=mybir.AxisListType.X, op=ALU.max
            )
            nc.vector.tensor_tensor(
                out=xch3,
                in0=xch3,
                in1=mx[:].unsqueeze(2).to_broadcast([128, nwc, H]),
                op=ALU.subtract,
            )
            enm = xs_pool.tile([128, nwc * H], F32, tag="enm")
            nc.scalar.activation(enm[:], xch[:], AF.Exp)
            ssum = xs_pool.tile([128, nwc], F32, tag="ssum")
            nc.vector.tensor_reduce(
                out=ssum[:],
                in_=enm[:].rearrange("p (w f) -> p w f", f=H),
                axis=mybir.AxisListType.X,
                op=ALU.add,
            )
            lsum = xs_pool.tile([128, nwc], F32, tag="lsum")
            nc.scalar.activation(lsum[:], ssum[:], AF.Ln)
            nc.vector.tensor_tensor(
                out=xch3,
                in0=xch3,
                in1=lsum[:].unsqueeze(2).to_broadcast([128, nwc, H]),
                op=ALU.subtract,
            )
            nc.sync.dma_start(
                out_view[:, cix * 4 : cix * 4 + nwc, :], xch3
            )

    nc.compile()
    return nc


# ---------------------------------------------------------------------------
# entry point
# ---------------------------------------------------------------------------


LAST_EXEC_NS = None
LAST_TRACE = None


def kernel(x, edge_index, edge_weight, W0, b0, W1, b1, conv_W):
    global LAST_EXEC_NS, LAST_TRACE
    from concourse.bass_utils import run_bass_kernel_spmd

    cfg = Cfg()
    in_maps, meta = preprocess(
        cfg, x, edge_index, edge_weight, W0, b0, W1, b1, conv_W
    )
    nc = build_program(cfg, meta)
    res = run_bass_kernel_spmd(nc, in_maps, list(range(cfg.n_cores)))
    LAST_EXEC_NS = getattr(res, "exec_time_ns", None)
    LAST_TRACE = getattr(res, "instructions_and_trace", None)
    outs = res.results
    full = np.concatenate(
        [np.asarray(outs[c]["out"])[: cfg.n_loc] for c in range(cfg.n_cores)],
        axis=0,
    )
    return full.astype(np.float32)



# revision 17
# speedup vs baseline: 2.3691x; 1.0059x over previous
"""GCNII (8-layer) graph-conv kernel for 8 TRN2 NeuronCores.

Node sharding: core c owns a contiguous block of destination nodes and all
edges into them. Per layer:
  gather   - dma_gather fetches each edge's source-feature row (256B) from a
             per-core DRAM replica of the full node table (bf16, feature dim
             padded 64->128 so rows are 256B = dma_gather granularity).
  scatter  - edges are grouped by 128-node dst windows; for each 128-edge
             tile one DVE tensor_scalar builds S[p,d] = (iota==dst_p)*w_p
             and the tensor engine accumulates msg^T @ S into a PSUM tile
             [64 feat x 128 dst].
  update   - support = psum + 0.1*x0 (the 0.9 edge scale is folded into the
             edge weights); h = relu(support @ Weff[l]) with
             Weff = (1-beta)I + beta*conv_W folded on the host.
  exchange - AllGather of each core's new h block rebuilds the table.

All cores run one SPMD program, so bucket sizes are equalized across cores on
the host (max over cores, rounded up to 128-edge tiles).
"""

import os
import sys

import numpy as np

for _p in ("/opt/trn_rl_repo", os.path.expanduser("~/trn_rl_repo")):
    if os.path.isdir(_p) and _p not in sys.path:
        sys.path.insert(0, _p)

import ml_dtypes  # noqa: E402

import concourse.bacc as bacc  # noqa: E402
import concourse.bass as bass  # noqa: E402
import concourse.mybir as mybir  # noqa: E402
import concourse.tile as tile  # noqa: E402

BF16 = mybir.dt.bfloat16
F32 = mybir.dt.float32
F32R = mybir.dt.float32r
I16 = mybir.dt.int16
NP_BF16 = ml_dtypes.bfloat16
AF = mybir.ActivationFunctionType
ALU = mybir.AluOpType


class Cfg:
    def __init__(
        self,
        n_nodes=100000,
        f_in=256,
        n_edges=3200000,
        n_layers=8,
        alpha=0.1,
        theta=0.5,
        n_cores=8,
        n_sb=4,
        group_w=2,
        use_for_i=False,  # collectives break inside Tile For_i loops
    ):
        self.n_nodes = n_nodes
        self.f_in = f_in
        self.h = 64
        self.n_edges = n_edges
        self.n_layers = n_layers
        self.alpha = alpha
        self.theta = theta
        self.n_cores = n_cores
        self.n_sb = n_sb
        self.group_w = group_w
        self.use_for_i = use_for_i

        assert n_nodes % n_cores == 0
        self.n_loc = n_nodes // n_cores
        self.n_win = (self.n_loc + 127) // 128
        self.n_loc_pad = self.n_win * 128
        self.n_tab = self.n_loc_pad * n_cores
        assert self.n_tab % n_sb == 0
        self.sb_rows = self.n_tab // n_sb
        assert self.sb_rows <= 32767, "int16 gather index limit"
        self.n_grp = (self.n_win + group_w - 1) // group_w
        assert f_in % 128 == 0

    def wins(self, g):
        return range(g * self.group_w, min((g + 1) * self.group_w, self.n_win))


# ---------------------------------------------------------------------------
# host preprocessing
# ---------------------------------------------------------------------------


def preprocess(cfg, x, edge_index, edge_weight, W0, b0, W1, b1, conv_W):
    C, H = cfg.n_cores, cfg.h
    src = np.asarray(edge_index[0], dtype=np.int64)
    dst = np.asarray(edge_index[1], dtype=np.int64)
    wgt = np.asarray(edge_weight, dtype=np.float32) * (1.0 - cfg.alpha)

    core = dst // cfg.n_loc
    loc = dst - core * cfg.n_loc
    win = loc // 128
    dl = (loc - win * 128).astype(NP_BF16)
    prow = src // cfg.n_loc * cfg.n_loc_pad + src % cfg.n_loc
    sb = prow // cfg.sb_rows
    rel = (prow - sb * cfg.sb_rows).astype(np.int16)

    key = (core * cfg.n_sb + sb) * cfg.n_win + win
    cnt = np.bincount(key, minlength=C * cfg.n_sb * cfg.n_win).reshape(
        C, cfg.n_sb, cfg.n_win
    )
    pad = ((cnt.max(axis=0) + 127) // 128) * 128  # [n_sb, n_win]
    ntile = pad // 128

    # stream layout: [g][sb][w in group] padded buckets
    bucket_off = np.zeros((cfg.n_sb, cfg.n_win), dtype=np.int64)
    chunk_base = np.zeros((cfg.n_grp, cfg.n_sb), dtype=np.int64)
    n_gsb = np.zeros((cfg.n_grp, cfg.n_sb), dtype=np.int64)
    idx_off = np.zeros((cfg.n_grp, cfg.n_sb), dtype=np.int64)
    off = 0
    for g in range(cfg.n_grp):
        for s in range(cfg.n_sb):
            chunk_base[g, s] = off
            idx_off[g, s] = off // 16
            for w in cfg.wins(g):
                bucket_off[s, w] = off
                off += int(pad[s, w])
            n_gsb[g, s] = off - chunk_base[g, s]
    tot_slots = off
    t_tot = tot_slots // 128

    order = np.lexsort((win, sb, core))
    beta = np.log(cfg.theta / (np.arange(cfg.n_layers) + 1) + 1.0).astype(
        np.float32
    )
    weff = np.zeros((H, cfg.n_layers * H), dtype=np.float32)
    for l in range(cfg.n_layers):
        weff[:, l * H : (l + 1) * H] = (1.0 - beta[l]) * np.eye(
            H, dtype=np.float32
        ) + beta[l] * np.asarray(conv_W[l], dtype=np.float32)

    x = np.asarray(x, dtype=np.float32)
    ntmax = int(n_gsb.max()) // 128
    consts = {
        "iotaw": np.tile(
            np.tile(np.arange(128, dtype=np.float32), ntmax), (128, 1)
        ).astype(NP_BF16),
        "w0": np.asarray(W0, dtype=np.float32),
        "b0": np.asarray(b0, dtype=np.float32).reshape(H, 1).copy(),
        "w1": np.asarray(W1, dtype=np.float32).astype(NP_BF16),
        "b1": np.asarray(b1, dtype=np.float32).reshape(H, 1).copy(),
        "weff": weff.astype(NP_BF16),
        "iota": np.tile(np.arange(128, dtype=np.float32), (128, 1)).astype(
            NP_BF16
        ),
        "identb": np.eye(128, dtype=np.float32).astype(NP_BF16),
        "identf": np.eye(128, dtype=np.float32),
    }

    in_maps = []
    for c in range(C):
        m = core[order] == c
        o = order[m]
        sb_c, win_c = sb[o], win[o]
        bkey = sb_c * cfg.n_win + win_c
        bc = np.bincount(bkey, minlength=cfg.n_sb * cfg.n_win)
        first = np.zeros(cfg.n_sb * cfg.n_win, dtype=np.int64)
        first[1:] = np.cumsum(bc)[:-1]
        rank = np.arange(bkey.size) - first[bkey]
        slot = bucket_off[sb_c, win_c] + rank

        idx_full = np.zeros(tot_slots, dtype=np.int16)
        dl_full = np.zeros(tot_slots, dtype=NP_BF16)
        w_full = np.zeros(tot_slots, dtype=NP_BF16)
        idx_full[slot] = rel[o]
        dl_full[slot] = dl[o]
        w_full[slot] = wgt[o].astype(NP_BF16)

        # gather index layout: [16, n/16] per region, tiled to 128 partitions
        idx_arr = np.tile(
            idx_full.reshape(tot_slots // 16, 16).T, (8, 1)
        )  # [128, tot_slots/16]

        # stream-tile order (no perm): column t = stream tile t
        dl_arr = np.ascontiguousarray(dl_full.reshape(t_tot, 128).T)
        w_arr = np.ascontiguousarray(w_full.reshape(t_tot, 128).T)

        xT = np.zeros((cfg.f_in, cfg.n_loc_pad), dtype=np.float32)
        xT[:, : cfg.n_loc] = x[c * cfg.n_loc : (c + 1) * cfg.n_loc].T

        in_maps.append(
            dict(consts, xT=xT, idx=idx_arr, dstc=dl_arr, wc=w_arr)
        )

    meta = dict(
        pad=pad,
        ntile=ntile,
        n_gsb=n_gsb,
        bucket_off=bucket_off,
        chunk_base=chunk_base,
        idx_off=idx_off,
        tot_slots=tot_slots,
        t_tot=t_tot,
        ntmax=ntmax,
    )
    return in_maps, meta


# ---------------------------------------------------------------------------
# device program
# ---------------------------------------------------------------------------


def build_program(cfg, meta):
    import contextlib

    H = cfg.h
    NW = cfg.n_win
    NSB = cfg.n_sb
    NLP = cfg.n_loc_pad
    ntile, n_gsb = meta["ntile"], meta["n_gsb"]
    kf = cfg.f_in // 128
    nchunk = (NLP + 511) // 512

    nc = bacc.Bacc("TRN2", target_bir_lowering=False, num_swdge_queues=4)

    xT_d = nc.declare_dram_parameter("xT", [cfg.f_in, NLP], F32, isOutput=False)
    idx_d = nc.declare_dram_parameter(
        "idx", [128, meta["tot_slots"] // 16], I16, isOutput=False
    )
    dstc_d = nc.declare_dram_parameter(
        "dstc", [128, meta["t_tot"]], BF16, isOutput=False
    )
    wc_d = nc.declare_dram_parameter("wc", [128, meta["t_tot"]], BF16, isOutput=False)
    w0_d = nc.declare_dram_parameter("w0", [cfg.f_in, H], F32, isOutput=False)
    b0_d = nc.declare_dram_parameter("b0", [H, 1], F32, isOutput=False)
    w1_d = nc.declare_dram_parameter("w1", [H, H], BF16, isOutput=False)
    b1_d = nc.declare_dram_parameter("b1", [H, 1], F32, isOutput=False)
    weff_d = nc.declare_dram_parameter(
        "weff", [H, cfg.n_layers * H], BF16, isOutput=False
    )
    iota_d = nc.declare_dram_parameter("iota", [128, 128], BF16, isOutput=False)
    iotaw_d = nc.declare_dram_parameter(
        "iotaw", [128, meta["ntmax"] * 128], BF16, isOutput=False
    )
    identb_d = nc.declare_dram_parameter("identb", [128, 128], BF16, isOutput=False)
    identf_d = nc.declare_dram_parameter("identf", [128, 128], F32, isOutput=False)
    out_d = nc.declare_dram_parameter("out", [NLP, H], F32, isOutput=True)

    from concourse import library_config

    with tile.TileContext(nc) as tc, contextlib.ExitStack() as ctx:
        nc.gpsimd.load_library(library_config.mlp)
        dram = ctx.enter_context(tc.tile_pool(name="dram", bufs=1, space="DRAM"))
        consts = ctx.enter_context(tc.tile_pool(name="consts", bufs=1))
        big = ctx.enter_context(tc.tile_pool(name="big", bufs=1))
        xs_pool = ctx.enter_context(tc.tile_pool(name="xs", bufs=3))
        idx_pool = ctx.enter_context(tc.tile_pool(name="idxp", bufs=2))
        dw_pool = ctx.enter_context(tc.tile_pool(name="dwp", bufs=2))
        msg_pools = [
            ctx.enter_context(tc.tile_pool(name=f"msg{s}", bufs=2))
            for s in range(NSB)
        ]
        sch_pools = [
            ctx.enter_context(tc.tile_pool(name=f"sch{s}", bufs=2))
            for s in range(NSB)
        ]
        tmp_pool = ctx.enter_context(tc.tile_pool(name="schtmp", bufs=2))
        stage_pool = ctx.enter_context(tc.tile_pool(name="stg", bufs=1))
        ps_agg = ctx.enter_context(tc.tile_pool(name="ps_agg", bufs=4, space="PSUM"))
        ps_big = ctx.enter_context(tc.tile_pool(name="ps_big", bufs=2, space="PSUM"))
        ps_tr = ctx.enter_context(tc.tile_pool(name="ps_tr", bufs=2, space="PSUM"))

        tables = [
            dram.tile(
                [cfg.n_tab, 128], BF16, addr_space="Shared", name=f"table{i}"
            )
            for i in range(cfg.n_layers + 1)
        ]
        staging = dram.tile([NLP, 128], BF16)

        iota_t = consts.tile([128, 128], BF16)
        iotaw_t = consts.tile([128, meta["ntmax"] * 128], BF16)
        identb_t = consts.tile([128, 128], BF16)
        identf_t = consts.tile([128, 128], F32)
        w0_t = consts.tile([128, kf * H], F32)
        w1_t = consts.tile([H, H], BF16)
        b0_t = consts.tile([H, 1], F32)
        b1_t = consts.tile([H, 1], F32)
        weff_t = consts.tile([H, cfg.n_layers * H], BF16)

        nc.sync.dma_start(iota_t[:], iota_d[:])
        nc.sync.dma_start(iotaw_t[:], iotaw_d[:])
        nc.sync.dma_start(identb_t[:], identb_d[:])
        nc.sync.dma_start(identf_t[:], identf_d[:])
        for k in range(kf):
            nc.sync.dma_start(
                w0_t[:, k * H : (k + 1) * H], w0_d[k * 128 : (k + 1) * 128, :]
            )
        nc.sync.dma_start(w1_t[:], w1_d[:])
        nc.sync.dma_start(b0_t[:], b0_d[:])
        nc.sync.dma_start(b1_t[:], b1_d[:])
        nc.sync.dma_start(weff_t[:], weff_d[:])

        x0s = big.tile([H, NLP], BF16)
        support = big.tile([H, NLP], BF16)
        hT = big.tile([H, NLP], BF16)
        stage_s = stage_pool.tile([128, NW * H], BF16)

        # zero the feature-pad half of the staging buffer, once (reuse
        # stage_s as the zero source; it is rewritten by every layer anyway)
        nc.vector.memset(stage_s[:], 0)
        nc.sync.dma_start(
            staging[:].rearrange("(w p) f -> p w f", p=128)[:, :, H:],
            stage_s[:].rearrange("p (w f) -> p w f", f=H),
        )

        # ---------------- prologue: h0 = relu(x @ W0 + b0) -----------------
        for cix in range(nchunk):
            c0 = cix * 512
            cw = min(512, NLP - c0)
            ps = ps_big.tile([H, 512], F32)
            for k in range(kf):
                xt = xs_pool.tile([128, 512], F32)
                nc.sync.dma_start(
                    xt[:, :cw], xT_d[k * 128 : (k + 1) * 128, c0 : c0 + cw]
                )
                nc.tensor.matmul(
                    ps[:, :cw],
                    lhsT=w0_t[:, k * H : (k + 1) * H],
                    rhs=xt[:, :cw],
                    start=(k == 0),
                    stop=(k == kf - 1),
                )
            nc.scalar.activation(
                hT[:, c0 : c0 + cw], ps[:, :cw], AF.Relu, bias=b0_t[:]
            )
            nc.vector.tensor_scalar_mul(
                x0s[:, c0 : c0 + cw], hT[:, c0 : c0 + cw], cfg.alpha
            )

        def stage_and_allgather(table):
            for w in range(NW):
                pst = ps_tr.tile([128, H], BF16)
                nc.tensor.transpose(
                    out=pst[:],
                    in_=hT[:, w * 128 : (w + 1) * 128],
                    identity=identb_t[:H, :H],
                )
                nc.vector.tensor_copy(
                    out=stage_s[:, w * H : (w + 1) * H], in_=pst[:]
                )
            nc.sync.dma_start(
                staging[:].rearrange("(w p) f -> p w f", p=128)[:, :, :H],
                stage_s[:].rearrange("p (w f) -> p w f", f=H),
            )
            nc.gpsimd.collective_compute(
                "AllGather",
                ALU.bypass,
                ins=[staging[:].opt()],
                outs=[table[:].opt()],
                replica_groups=[list(range(cfg.n_cores))],
            )

        stage_and_allgather(tables[0])

        # ---------------- per-layer body -----------------------------------
        def layer_body(li):
            table = tables[li]
            nc.vector.tensor_copy(out=support[:], in_=x0s[:])
            for g in range(cfg.n_grp):
                tg = int(n_gsb[g].sum()) // 128
                t0g = int(meta["chunk_base"][g, 0]) // 128
                idxn = int(n_gsb[g].sum()) // 16
                if idxn:
                    idx_t = idx_pool.tile([128, idxn], I16)
                    nc.sync.dma_start(
                        idx_t[:],
                        idx_d[:, int(meta["idx_off"][g, 0]) :][:, :idxn],
                    )
                if tg:
                    dst_t = dw_pool.tile([128, tg], BF16, tag="dst")
                    w_t = dw_pool.tile([128, tg], BF16, tag="w")
                    nc.sync.dma_start(dst_t[:], dstc_d[:, t0g : t0g + tg])
                    nc.sync.dma_start(w_t[:], wc_d[:, t0g : t0g + tg])

                msgs = {}
                schs = {}
                for s in range(NSB):
                    n = int(n_gsb[g, s])
                    if n == 0:
                        continue
                    nt = n // 128
                    mt = msg_pools[s].tile([128, nt, 128], BF16)
                    ic = int(meta["idx_off"][g, s] - meta["idx_off"][g, 0])
                    nc.gpsimd.dma_gather(
                        mt[:],
                        table[s * cfg.sb_rows : (s + 1) * cfg.sb_rows, :],
                        idx_t[:, ic : ic + n // 16],
                        n,
                        n,
                        128,
                        single_packet=False,  # single-packet mode breaks >1k idxs
                        queue_num=s,
                    )
                    msgs[s] = mt
                for s in range(NSB):
                    n = int(n_gsb[g, s])
                    if n == 0:
                        continue
                    nt = n // 128
                    # S chunk for (g, s): S[p, t, d] = (d == dst[p,t]) * w[p,t]
                    # built with tensor_tensor (never takes the 2-port SBUF
                    # lock, so SWDGE descriptor generation is not starved)
                    c0 = int(
                        meta["chunk_base"][g, s] - meta["chunk_base"][g, 0]
                    ) // 128
                    sch = sch_pools[s].tile([128, nt, 128], BF16)
                    tmp = tmp_pool.tile([128, nt, 128], BF16)
                    iow = iotaw_t[:, : nt * 128].rearrange(
                        "p (t d) -> p t d", d=128
                    )
                    nc.vector.tensor_tensor(
                        out=tmp[:],
                        in0=iow,
                        in1=dst_t[:, c0 : c0 + nt]
                        .unsqueeze(2)
                        .to_broadcast([128, nt, 128]),
                        op=ALU.is_equal,
                    )
                    nc.vector.tensor_tensor(
                        out=sch[:],
                        in0=tmp[:],
                        in1=w_t[:, c0 : c0 + nt]
                        .unsqueeze(2)
                        .to_broadcast([128, nt, 128]),
                        op=ALU.mult,
                    )
                    schs[s] = sch

                for w in cfg.wins(g):
                    kt = int(sum(ntile[s, w] for s in range(NSB)))
                    if kt == 0:
                        continue
                    ps = ps_agg.tile([H, 128], F32)
                    k = 0
                    for s in range(NSB):
                        nt = int(ntile[s, w])
                        if nt == 0:
                            continue
                        i0 = int(
                            meta["bucket_off"][s, w] - meta["chunk_base"][g, s]
                        ) // 128
                        for i in range(nt):
                            nc.tensor.matmul(
                                ps[:],
                                lhsT=msgs[s][:, i0 + i, :H],
                                rhs=schs[s][:, i0 + i, :],
                                start=(k == 0),
                                stop=(k == kt - 1),
                            )
                            k += 1
                    nc.vector.tensor_tensor(
                        out=support[:, w * 128 : (w + 1) * 128],
                        in0=support[:, w * 128 : (w + 1) * 128],
                        in1=ps[:],
                        op=ALU.add,
                    )

            if isinstance(li, int):
                wsl = weff_t[:, li * H : (li + 1) * H]
            else:
                # walrus can't take a register offset in ldweights: copy the
                # dynamic Weff slice to a fixed location first
                wsl_t = consts.tile([H, H], BF16, tag="wsl")
                nc.vector.tensor_copy(
                    out=wsl_t[:], in_=weff_t[:, bass.ds(li * H, H)]
                )
                wsl = wsl_t[:]
            for cix in range(nchunk):
                c0 = cix * 512
                cw = min(512, NLP - c0)
                ps = ps_big.tile([H, 512], F32)
                nc.tensor.matmul(
                    ps[:, :cw],
                    lhsT=wsl,
                    rhs=support[:, c0 : c0 + cw],
                    start=True,
                    stop=True,
                )
                nc.scalar.activation(hT[:, c0 : c0 + cw], ps[:, :cw], AF.Relu)
            stage_and_allgather(tables[li + 1])

        if cfg.use_for_i:
            with tc.For_i(0, cfg.n_layers, 1) as li:
                layer_body(li)
        else:
            for li in range(cfg.n_layers):
                layer_body(li)

        # ---------------- epilogue: logits + log_softmax, chunk-wise --------
        out_view = out_d[:].rearrange("(w p) f -> p w f", p=128)
        for cix in range(nchunk):
            c0 = cix * 512
            cw = min(512, NLP - c0)
            nwc = cw // 128  # windows in this chunk
            ps = ps_big.tile([H, 512], F32)
            nc.tensor.matmul(
                ps[:, :cw], lhsT=w1_t[:], rhs=hT[:, c0 : c0 + cw],
                start=True, stop=True,
            )
            lt = xs_pool.tile([H, 512], F32, tag="lt")
            nc.scalar.activation(
                lt[:, :cw], ps[:, :cw], AF.Identity, bias=b1_t[:]
            )
            xch = xs_pool.tile([128, nwc * H], F32, tag="xch")
            for wi in range(nwc):
                pst = ps_tr.tile([128, H], F32)
                nc.tensor.transpose(
                    out=pst[:],
                    in_=lt[:, wi * 128 : (wi + 1) * 128],
                    identity=identf_t[:H, :H],
                )
                nc.vector.tensor_copy(
                    out=xch[:, wi * H : (wi + 1) * H], in_=pst[:]
                )
            xch3 = xch[:].rearrange("p (w f) -> p w f", f=H)
            mx = xs_pool.tile([128, nwc], F32, tag="mx")
            nc.vector.tensor_reduce(
                out=mx[:], in_=xch3, axis=mybir.AxisListType.X, op=ALU.max
            )
            nc.vector.tensor_tensor(
                out=xch3,
                in0=xch3,
                in1=mx[:].unsqueeze(2).to_broadcast([128, nwc, H]),
                op=ALU.subtract,
            )
            enm = xs_pool.tile([128, nwc * H], F32, tag="enm")
            nc.scalar.activation(enm[:], xch[:], AF.Exp)
            ssum = xs_pool.tile([128, nwc], F32, tag="ssum")
            nc.vector.tensor_reduce(
                out=ssum[:],
                in_=enm[:].rearrange("p (w f) -> p w f", f=H),
                axis=mybir.AxisListType.X,
                op=ALU.add,
            )
            lsum = xs_pool.tile([128, nwc], F32, tag="lsum")
            nc.scalar.activation(lsum[:], ssum[:], AF.Ln)
            nc.vector.tensor_tensor(
                out=xch3,
                in0=xch3,
                in1=lsum[:].unsqueeze(2).to_broadcast([128, nwc, H]),
                op=ALU.subtract,
            )
            nc.sync.dma_start(
                out_view[:, cix * 4 : cix * 4 + nwc, :], xch3
            )

    nc.compile()
    return nc


# ---------------------------------------------------------------------------
# entry point
# ---------------------------------------------------------------------------


LAST_EXEC_NS = None
LAST_TRACE = None


def kernel(x, edge_index, edge_weight, W0, b0, W1, b1, conv_W):
    global LAST_EXEC_NS, LAST_TRACE
    from concourse.bass_utils import run_bass_kernel_spmd

    cfg = Cfg()
    in_maps, meta = preprocess(
        cfg, x, edge_index, edge_weight, W0, b0, W1, b1, conv_W
    )
    nc = build_program(cfg, meta)
    res = run_bass_kernel_spmd(nc, in_maps, list(range(cfg.n_cores)))
    LAST_EXEC_NS = getattr(res, "exec_time_ns", None)
    LAST_TRACE = getattr(res, "instructions_and_trace", None)
    outs = res.results
    full = np.concatenate(
        [np.asarray(outs[c]["out"])[: cfg.n_loc] for c in range(cfg.n_cores)],
        axis=0,
    )
    return full.astype(np.float32)



# revision 28
# speedup vs baseline: 2.4545x; 1.0360x over previous
"""GCNII (8-layer) graph-conv kernel for 8 TRN2 NeuronCores.

Node sharding: core c owns a contiguous block of destination nodes and all
edges into them. Per layer:
  gather   - dma_gather fetches each edge's source-feature row (256B) from a
             per-core DRAM replica of the full node table (bf16, feature dim
             padded 64->128 so rows are 256B = dma_gather granularity).
  scatter  - edges are grouped by 128-node dst windows; for each 128-edge
             tile one DVE tensor_scalar builds S[p,d] = (iota==dst_p)*w_p
             and the tensor engine accumulates msg^T @ S into a PSUM tile
             [64 feat x 128 dst].
  update   - support = psum + 0.1*x0 (the 0.9 edge scale is folded into the
             edge weights); h = relu(support @ Weff[l]) with
             Weff = (1-beta)I + beta*conv_W folded on the host.
  exchange - AllGather of each core's new h block rebuilds the table.

All cores run one SPMD program, so bucket sizes are equalized across cores on
the host (max over cores, rounded up to 128-edge tiles).
"""

import os
import sys

import numpy as np

for _p in ("/opt/trn_rl_repo", os.path.expanduser("~/trn_rl_repo")):
    if os.path.isdir(_p) and _p not in sys.path:
        sys.path.insert(0, _p)

import ml_dtypes  # noqa: E402

import concourse.bacc as bacc  # noqa: E402
import concourse.bass as bass  # noqa: E402
import concourse.mybir as mybir  # noqa: E402
import concourse.tile as tile  # noqa: E402

BF16 = mybir.dt.bfloat16
F32 = mybir.dt.float32
F32R = mybir.dt.float32r
I16 = mybir.dt.int16
NP_BF16 = ml_dtypes.bfloat16
AF = mybir.ActivationFunctionType
ALU = mybir.AluOpType


class Cfg:
    def __init__(
        self,
        n_nodes=100000,
        f_in=256,
        n_edges=3200000,
        n_layers=8,
        alpha=0.1,
        theta=0.5,
        n_cores=8,
        n_sb=4,
        group_w=2,
        use_for_i=False,  # collectives break inside Tile For_i loops
    ):
        self.n_nodes = n_nodes
        self.f_in = f_in
        self.h = 64
        self.n_edges = n_edges
        self.n_layers = n_layers
        self.alpha = alpha
        self.theta = theta
        self.n_cores = n_cores
        self.n_sb = n_sb
        self.group_w = group_w
        self.use_for_i = use_for_i

        assert n_nodes % n_cores == 0
        self.n_loc = n_nodes // n_cores
        self.n_win = (self.n_loc + 127) // 128
        self.n_loc_pad = self.n_win * 128
        self.n_tab = self.n_loc_pad * n_cores
        assert self.n_tab % n_sb == 0
        self.sb_rows = self.n_tab // n_sb
        assert self.sb_rows <= 32767, "int16 gather index limit"
        self.n_grp = (self.n_win + group_w - 1) // group_w
        assert f_in % 128 == 0

    def wins(self, g):
        return range(g * self.group_w, min((g + 1) * self.group_w, self.n_win))


# ---------------------------------------------------------------------------
# host preprocessing
# ---------------------------------------------------------------------------


def preprocess(cfg, x, edge_index, edge_weight, W0, b0, W1, b1, conv_W):
    C, H = cfg.n_cores, cfg.h
    src = np.asarray(edge_index[0], dtype=np.int64)
    dst = np.asarray(edge_index[1], dtype=np.int64)
    wgt = np.asarray(edge_weight, dtype=np.float32) * (1.0 - cfg.alpha)

    core = dst // cfg.n_loc
    loc = dst - core * cfg.n_loc
    win = loc // 128
    dl = (loc - win * 128).astype(NP_BF16)
    prow = src // cfg.n_loc * cfg.n_loc_pad + src % cfg.n_loc
    sb = prow // cfg.sb_rows
    rel = (prow - sb * cfg.sb_rows).astype(np.int16)

    key = (core * cfg.n_sb + sb) * cfg.n_win + win
    cnt = np.bincount(key, minlength=C * cfg.n_sb * cfg.n_win).reshape(
        C, cfg.n_sb, cfg.n_win
    )
    pad = ((cnt.max(axis=0) + 127) // 128) * 128  # [n_sb, n_win]
    ntile = pad // 128

    # stream layout: [g][sb][w in group] padded buckets
    bucket_off = np.zeros((cfg.n_sb, cfg.n_win), dtype=np.int64)
    chunk_base = np.zeros((cfg.n_grp, cfg.n_sb), dtype=np.int64)
    n_gsb = np.zeros((cfg.n_grp, cfg.n_sb), dtype=np.int64)
    idx_off = np.zeros((cfg.n_grp, cfg.n_sb), dtype=np.int64)
    off = 0
    for g in range(cfg.n_grp):
        for s in range(cfg.n_sb):
            chunk_base[g, s] = off
            idx_off[g, s] = off // 16
            for w in cfg.wins(g):
                bucket_off[s, w] = off
                off += int(pad[s, w])
            n_gsb[g, s] = off - chunk_base[g, s]
    tot_slots = off
    t_tot = tot_slots // 128

    order = np.lexsort((win, sb, core))
    beta = np.log(cfg.theta / (np.arange(cfg.n_layers) + 1) + 1.0).astype(
        np.float32
    )
    weff = np.zeros((H, cfg.n_layers * H), dtype=np.float32)
    for l in range(cfg.n_layers):
        weff[:, l * H : (l + 1) * H] = (1.0 - beta[l]) * np.eye(
            H, dtype=np.float32
        ) + beta[l] * np.asarray(conv_W[l], dtype=np.float32)

    x = np.asarray(x, dtype=np.float32)
    ntmax = int(n_gsb.max()) // 128
    consts = {
        "w0": np.asarray(W0, dtype=np.float32),
        "b0": np.asarray(b0, dtype=np.float32).reshape(H, 1).copy(),
        "w1": np.asarray(W1, dtype=np.float32).astype(NP_BF16),
        "b1": np.asarray(b1, dtype=np.float32).reshape(H, 1).copy(),
        "weff": weff.astype(NP_BF16),
        "iota": np.tile(np.arange(128, dtype=np.float32), (128, 1)).astype(
            NP_BF16
        ),
        "identb": np.eye(128, dtype=np.float32).astype(NP_BF16),
        "identf": np.eye(128, dtype=np.float32),
    }

    in_maps = []
    for c in range(C):
        m = core[order] == c
        o = order[m]
        sb_c, win_c = sb[o], win[o]
        bkey = sb_c * cfg.n_win + win_c
        bc = np.bincount(bkey, minlength=cfg.n_sb * cfg.n_win)
        first = np.zeros(cfg.n_sb * cfg.n_win, dtype=np.int64)
        first[1:] = np.cumsum(bc)[:-1]
        rank = np.arange(bkey.size) - first[bkey]
        slot = bucket_off[sb_c, win_c] + rank

        idx_full = np.zeros(tot_slots, dtype=np.int16)
        dl_full = np.zeros(tot_slots, dtype=NP_BF16)
        w_full = np.zeros(tot_slots, dtype=NP_BF16)
        idx_full[slot] = rel[o]
        dl_full[slot] = dl[o]
        w_full[slot] = wgt[o].astype(NP_BF16)

        # gather index layout: [16, n/16] per region, tiled to 128 partitions
        idx_arr = np.tile(
            idx_full.reshape(tot_slots // 16, 16).T, (8, 1)
        )  # [128, tot_slots/16]

        # host-precomputed scatter matrix S (layer-invariant):
        # S[p, t, d] = (d == dst_rel[p,t]) * w[p,t], stream-tile order.
        dl_arr = dl_full.reshape(t_tot, 128).T
        w_arr = w_full.reshape(t_tot, 128).T
        sc_arr = (
            (
                dl_arr[:, :, None].astype(np.float32)
                == np.arange(128, dtype=np.float32)[None, None, :]
            )
            * w_arr[:, :, None].astype(np.float32)
        ).astype(NP_BF16).reshape(128, t_tot * 128)

        xT = np.zeros((cfg.f_in, cfg.n_loc_pad), dtype=np.float32)
        xT[:, : cfg.n_loc] = x[c * cfg.n_loc : (c + 1) * cfg.n_loc].T

        in_maps.append(dict(consts, xT=xT, idx=idx_arr, sc=sc_arr))

    meta = dict(
        pad=pad,
        ntile=ntile,
        n_gsb=n_gsb,
        bucket_off=bucket_off,
        chunk_base=chunk_base,
        idx_off=idx_off,
        tot_slots=tot_slots,
        t_tot=t_tot,
        ntmax=ntmax,
    )
    return in_maps, meta


# ---------------------------------------------------------------------------
# device program
# ---------------------------------------------------------------------------


def build_program(cfg, meta):
    import contextlib

    H = cfg.h
    NW = cfg.n_win
    NSB = cfg.n_sb
    NLP = cfg.n_loc_pad
    ntile, n_gsb = meta["ntile"], meta["n_gsb"]
    kf = cfg.f_in // 128
    nchunk = (NLP + 511) // 512

    nc = bacc.Bacc("TRN2", target_bir_lowering=False, num_swdge_queues=4)

    xT_d = nc.declare_dram_parameter("xT", [cfg.f_in, NLP], F32, isOutput=False)
    idx_d = nc.declare_dram_parameter(
        "idx", [128, meta["tot_slots"] // 16], I16, isOutput=False
    )
    sc_d = nc.declare_dram_parameter(
        "sc", [128, meta["t_tot"] * 128], BF16, isOutput=False
    )
    w0_d = nc.declare_dram_parameter("w0", [cfg.f_in, H], F32, isOutput=False)
    b0_d = nc.declare_dram_parameter("b0", [H, 1], F32, isOutput=False)
    w1_d = nc.declare_dram_parameter("w1", [H, H], BF16, isOutput=False)
    b1_d = nc.declare_dram_parameter("b1", [H, 1], F32, isOutput=False)
    weff_d = nc.declare_dram_parameter(
        "weff", [H, cfg.n_layers * H], BF16, isOutput=False
    )
    iota_d = nc.declare_dram_parameter("iota", [128, 128], BF16, isOutput=False)
    identb_d = nc.declare_dram_parameter("identb", [128, 128], BF16, isOutput=False)
    identf_d = nc.declare_dram_parameter("identf", [128, 128], F32, isOutput=False)
    out_d = nc.declare_dram_parameter("out", [NLP, H], F32, isOutput=True)

    from concourse import library_config

    with tile.TileContext(nc) as tc, contextlib.ExitStack() as ctx:
        nc.gpsimd.load_library(library_config.mlp)
        dram = ctx.enter_context(tc.tile_pool(name="dram", bufs=1, space="DRAM"))
        consts = ctx.enter_context(tc.tile_pool(name="consts", bufs=1))
        big = ctx.enter_context(tc.tile_pool(name="big", bufs=1))
        xs_pool = ctx.enter_context(tc.tile_pool(name="xs", bufs=3))
        idx_pool = ctx.enter_context(tc.tile_pool(name="idxp", bufs=2))
        dw_pool = ctx.enter_context(tc.tile_pool(name="dwp", bufs=2))
        msg_pools = [
            ctx.enter_context(tc.tile_pool(name=f"msg{s}", bufs=2))
            for s in range(NSB)
        ]
        sch_pools = [
            ctx.enter_context(tc.tile_pool(name=f"sch{s}", bufs=2))
            for s in range(NSB)
        ]
        stage_pool = ctx.enter_context(tc.tile_pool(name="stg", bufs=1))
        ps_agg = ctx.enter_context(tc.tile_pool(name="ps_agg", bufs=4, space="PSUM"))
        ps_big = ctx.enter_context(tc.tile_pool(name="ps_big", bufs=2, space="PSUM"))
        ps_tr = ctx.enter_context(tc.tile_pool(name="ps_tr", bufs=2, space="PSUM"))

        tables = [
            dram.tile(
                [cfg.n_tab, 128], BF16, addr_space="Shared", name=f"table{i}"
            )
            for i in range(cfg.n_layers + 1)
        ]
        staging = dram.tile([NLP, 128], BF16)

        iota_t = consts.tile([128, 128], BF16)
        identb_t = consts.tile([128, 128], BF16)
        identf_t = consts.tile([128, 128], F32)
        w0_t = consts.tile([128, kf * H], F32)
        w1_t = consts.tile([H, H], BF16)
        b0_t = consts.tile([H, 1], F32)
        b1_t = consts.tile([H, 1], F32)
        weff_t = consts.tile([H, cfg.n_layers * H], BF16)

        nc.sync.dma_start(iota_t[:], iota_d[:])
        nc.sync.dma_start(identb_t[:], identb_d[:])
        nc.sync.dma_start(identf_t[:], identf_d[:])
        for k in range(kf):
            nc.sync.dma_start(
                w0_t[:, k * H : (k + 1) * H], w0_d[k * 128 : (k + 1) * 128, :]
            )
        nc.sync.dma_start(w1_t[:], w1_d[:])
        nc.sync.dma_start(b0_t[:], b0_d[:])
        nc.sync.dma_start(b1_t[:], b1_d[:])
        nc.sync.dma_start(weff_t[:], weff_d[:])

        x0s = big.tile([H, NLP], BF16)
        support = big.tile([H, NLP], BF16)
        hT = big.tile([H, NLP], BF16)
        stage_s = stage_pool.tile([128, NW * H], BF16)

        # zero the feature-pad half of the staging buffer, once (reuse
        # stage_s as the zero source; it is rewritten by every layer anyway)
        nc.vector.memset(stage_s[:], 0)
        nc.sync.dma_start(
            staging[:].rearrange("(w p) f -> p w f", p=128)[:, :, H:],
            stage_s[:].rearrange("p (w f) -> p w f", f=H),
        )

        # ---------------- prologue: h0 = relu(x @ W0 + b0) -----------------
        for cix in range(nchunk):
            c0 = cix * 512
            cw = min(512, NLP - c0)
            ps = ps_big.tile([H, 512], F32)
            for k in range(kf):
                xt = xs_pool.tile([128, 512], F32)
                nc.sync.dma_start(
                    xt[:, :cw], xT_d[k * 128 : (k + 1) * 128, c0 : c0 + cw]
                )
                nc.tensor.matmul(
                    ps[:, :cw],
                    lhsT=w0_t[:, k * H : (k + 1) * H],
                    rhs=xt[:, :cw],
                    start=(k == 0),
                    stop=(k == kf - 1),
                )
            nc.scalar.activation(
                hT[:, c0 : c0 + cw], ps[:, :cw], AF.Relu, bias=b0_t[:]
            )
            nc.vector.tensor_scalar_mul(
                x0s[:, c0 : c0 + cw], hT[:, c0 : c0 + cw], cfg.alpha
            )

        def stage_and_allgather(table):
            for w in range(NW):
                pst = ps_tr.tile([128, H], BF16)
                nc.tensor.transpose(
                    out=pst[:],
                    in_=hT[:, w * 128 : (w + 1) * 128],
                    identity=identb_t[:H, :H],
                )
                nc.vector.tensor_copy(
                    out=stage_s[:, w * H : (w + 1) * H], in_=pst[:]
                )
            nc.sync.dma_start(
                staging[:].rearrange("(w p) f -> p w f", p=128)[:, :, :H],
                stage_s[:].rearrange("p (w f) -> p w f", f=H),
            )
            nc.gpsimd.collective_compute(
                "AllGather",
                ALU.bypass,
                ins=[staging[:].opt()],
                outs=[table[:].opt()],
                replica_groups=[list(range(cfg.n_cores))],
            )

        stage_and_allgather(tables[0])

        # ---------------- per-layer body -----------------------------------
        def layer_body(li):
            table = tables[li]
            nc.vector.tensor_copy(out=support[:], in_=x0s[:])
            for g in range(cfg.n_grp):
                tg = int(n_gsb[g].sum()) // 128
                t0g = int(meta["chunk_base"][g, 0]) // 128
                idxn = int(n_gsb[g].sum()) // 16
                if idxn:
                    idx_t = idx_pool.tile([128, idxn], I16)
                    nc.sync.dma_start(
                        idx_t[:],
                        idx_d[:, int(meta["idx_off"][g, 0]) :][:, :idxn],
                    )
                msgs = {}
                schs = {}
                for s in range(NSB):
                    n = int(n_gsb[g, s])
                    if n == 0:
                        continue
                    nt = n // 128
                    mt = msg_pools[s].tile([128, nt, 128], BF16)
                    ic = int(meta["idx_off"][g, s] - meta["idx_off"][g, 0])
                    nc.gpsimd.dma_gather(
                        mt[:],
                        table[s * cfg.sb_rows : (s + 1) * cfg.sb_rows, :],
                        idx_t[:, ic : ic + n // 16],
                        n,
                        n,
                        128,
                        single_packet=False,  # single-packet mode breaks >1k idxs
                        queue_num=s,
                    )
                    msgs[s] = mt
                for s in range(NSB):
                    n = int(n_gsb[g, s])
                    if n == 0:
                        continue
                    nt = n // 128
                    # S chunk for (g, s): host-precomputed (layer-invariant),
                    # streamed sequentially from DRAM — no DVE work at all
                    t0 = int(meta["chunk_base"][g, s]) // 128
                    sch = sch_pools[s].tile([128, nt, 128], BF16)
                    nc.sync.dma_start(
                        sch[:].rearrange("p t d -> p (t d)"),
                        sc_d[:, t0 * 128 : (t0 + nt) * 128],
                    )
                    schs[s] = sch

                for w in cfg.wins(g):
                    kt = int(sum(ntile[s, w] for s in range(NSB)))
                    if kt == 0:
                        continue
                    ps = ps_agg.tile([H, 128], F32)
                    k = 0
                    for s in range(NSB):
                        nt = int(ntile[s, w])
                        if nt == 0:
                            continue
                        i0 = int(
                            meta["bucket_off"][s, w] - meta["chunk_base"][g, s]
                        ) // 128
                        for i in range(nt):
                            nc.tensor.matmul(
                                ps[:],
                                lhsT=msgs[s][:, i0 + i, :H],
                                rhs=schs[s][:, i0 + i, :],
                                start=(k == 0),
                                stop=(k == kt - 1),
                            )
                            k += 1
                    nc.vector.tensor_tensor(
                        out=support[:, w * 128 : (w + 1) * 128],
                        in0=support[:, w * 128 : (w + 1) * 128],
                        in1=ps[:],
                        op=ALU.add,
                    )

            if isinstance(li, int):
                wsl = weff_t[:, li * H : (li + 1) * H]
            else:
                # walrus can't take a register offset in ldweights: copy the
                # dynamic Weff slice to a fixed location first
                wsl_t = consts.tile([H, H], BF16, tag="wsl")
                nc.vector.tensor_copy(
                    out=wsl_t[:], in_=weff_t[:, bass.ds(li * H, H)]
                )
                wsl = wsl_t[:]
            for cix in range(nchunk):
                c0 = cix * 512
                cw = min(512, NLP - c0)
                ps = ps_big.tile([H, 512], F32)
                nc.tensor.matmul(
                    ps[:, :cw],
                    lhsT=wsl,
                    rhs=support[:, c0 : c0 + cw],
                    start=True,
                    stop=True,
                )
                nc.scalar.activation(hT[:, c0 : c0 + cw], ps[:, :cw], AF.Relu)
            stage_and_allgather(tables[li + 1])

        if cfg.use_for_i:
            with tc.For_i(0, cfg.n_layers, 1) as li:
                layer_body(li)
        else:
            for li in range(cfg.n_layers):
                layer_body(li)

        # ---------------- epilogue: logits + log_softmax, chunk-wise --------
        out_view = out_d[:].rearrange("(w p) f -> p w f", p=128)
        for cix in range(nchunk):
            c0 = cix * 512
            cw = min(512, NLP - c0)
            nwc = cw // 128  # windows in this chunk
            ps = ps_big.tile([H, 512], F32)
            nc.tensor.matmul(
                ps[:, :cw], lhsT=w1_t[:], rhs=hT[:, c0 : c0 + cw],
                start=True, stop=True,
            )
            lt = xs_pool.tile([H, 512], F32, tag="lt")
            nc.scalar.activation(
                lt[:, :cw], ps[:, :cw], AF.Identity, bias=b1_t[:]
            )
            xch = xs_pool.tile([128, nwc * H], F32, tag="xch")
            for wi in range(nwc):
                pst = ps_tr.tile([128, H], F32)
                nc.tensor.transpose(
                    out=pst[:],
                    in_=lt[:, wi * 128 : (wi + 1) * 128],
                    identity=identf_t[:H, :H],
                )
                nc.vector.tensor_copy(
                    out=xch[:, wi * H : (wi + 1) * H], in_=pst[:]
                )
            xch3 = xch[:].rearrange("p (w f) -> p w f", f=H)
            mx = xs_pool.tile([128, nwc], F32, tag="mx")
            nc.vector.tensor_reduce(
                out=mx[:], in_=xch3, axis=mybir.AxisListType.X, op=ALU.max
            )
            nc.vector.tensor_tensor(
                out=xch3,
                in0=xch3,
                in1=mx[:].unsqueeze(2).to_broadcast([128, nwc, H]),
                op=ALU.subtract,
            )
            enm = xs_pool.tile([128, nwc * H], F32, tag="enm")
            nc.scalar.activation(enm[:], xch[:], AF.Exp)
            ssum = xs_pool.tile([128, nwc], F32, tag="ssum")
            nc.vector.tensor_reduce(
                out=ssum[:],
                in_=enm[:].rearrange("p (w f) -> p w f", f=H),
                axis=mybir.AxisListType.X,
                op=ALU.add,
            )
            lsum = xs_pool.tile([128, nwc], F32, tag="lsum")
            nc.scalar.activation(lsum[:], ssum[:], AF.Ln)
            nc.vector.tensor_tensor(
                out=xch3,
                in0=xch3,
                in1=lsum[:].unsqueeze(2).to_broadcast([128, nwc, H]),
                op=ALU.subtract,
            )
            nc.sync.dma_start(
                out_view[:, cix * 4 : cix * 4 + nwc, :], xch3
            )

    nc.compile()
    return nc


# ---------------------------------------------------------------------------
# entry point
# ---------------------------------------------------------------------------


LAST_EXEC_NS = None
LAST_TRACE = None


def kernel(x, edge_index, edge_weight, W0, b0, W1, b1, conv_W):
    global LAST_EXEC_NS, LAST_TRACE
    from concourse.bass_utils import run_bass_kernel_spmd

    cfg = Cfg()
    in_maps, meta = preprocess(
        cfg, x, edge_index, edge_weight, W0, b0, W1, b1, conv_W
    )
    nc = build_program(cfg, meta)
    res = run_bass_kernel_spmd(nc, in_maps, list(range(cfg.n_cores)))
    LAST_EXEC_NS = getattr(res, "exec_time_ns", None)
    LAST_TRACE = getattr(res, "instructions_and_trace", None)
    outs = res.results
    full = np.concatenate(
        [np.asarray(outs[c]["out"])[: cfg.n_loc] for c in range(cfg.n_cores)],
        axis=0,
    )
    return full.astype(np.float32)



# revision 30
# speedup vs baseline: 2.6929x; 1.0971x over previous
"""GCNII (8-layer) graph-conv kernel for 8 TRN2 NeuronCores.

Node sharding: core c owns a contiguous block of destination nodes and all
edges into them. Per layer:
  gather   - dma_gather fetches each edge's source-feature row (256B) from a
             per-core DRAM replica of the full node table (bf16, feature dim
             padded 64->128 so rows are 256B = dma_gather granularity).
  scatter  - edges are grouped by 128-node dst windows; for each 128-edge
             tile one DVE tensor_scalar builds S[p,d] = (iota==dst_p)*w_p
             and the tensor engine accumulates msg^T @ S into a PSUM tile
             [64 feat x 128 dst].
  update   - support = psum + 0.1*x0 (the 0.9 edge scale is folded into the
             edge weights); h = relu(support @ Weff[l]) with
             Weff = (1-beta)I + beta*conv_W folded on the host.
  exchange - AllGather of each core's new h block rebuilds the table.

All cores run one SPMD program, so bucket sizes are equalized across cores on
the host (max over cores, rounded up to 128-edge tiles).
"""

import os
import sys

import numpy as np

for _p in ("/opt/trn_rl_repo", os.path.expanduser("~/trn_rl_repo")):
    if os.path.isdir(_p) and _p not in sys.path:
        sys.path.insert(0, _p)

import ml_dtypes  # noqa: E402

import concourse.bacc as bacc  # noqa: E402
import concourse.bass as bass  # noqa: E402
import concourse.mybir as mybir  # noqa: E402
import concourse.tile as tile  # noqa: E402

BF16 = mybir.dt.bfloat16
F32 = mybir.dt.float32
F32R = mybir.dt.float32r
I16 = mybir.dt.int16
NP_BF16 = ml_dtypes.bfloat16
AF = mybir.ActivationFunctionType
ALU = mybir.AluOpType


class Cfg:
    def __init__(
        self,
        n_nodes=100000,
        f_in=256,
        n_edges=3200000,
        n_layers=8,
        alpha=0.1,
        theta=0.5,
        n_cores=8,
        n_sb=4,
        group_w=2,
        use_for_i=False,  # collectives break inside Tile For_i loops
    ):
        self.n_nodes = n_nodes
        self.f_in = f_in
        self.h = 64
        self.n_edges = n_edges
        self.n_layers = n_layers
        self.alpha = alpha
        self.theta = theta
        self.n_cores = n_cores
        self.n_sb = n_sb
        self.group_w = group_w
        self.use_for_i = use_for_i

        assert n_nodes % n_cores == 0
        self.n_loc = n_nodes // n_cores
        self.n_win = (self.n_loc + 127) // 128
        self.n_loc_pad = self.n_win * 128
        self.n_tab = self.n_loc_pad * n_cores
        assert self.n_tab % n_sb == 0
        self.sb_rows = self.n_tab // n_sb
        assert self.sb_rows <= 32767, "int16 gather index limit"
        self.n_grp = (self.n_win + group_w - 1) // group_w
        assert f_in % 128 == 0

    def wins(self, g):
        return range(g * self.group_w, min((g + 1) * self.group_w, self.n_win))


# ---------------------------------------------------------------------------
# host preprocessing
# ---------------------------------------------------------------------------


def preprocess(cfg, x, edge_index, edge_weight, W0, b0, W1, b1, conv_W):
    C, H = cfg.n_cores, cfg.h
    src = np.asarray(edge_index[0], dtype=np.int64)
    dst = np.asarray(edge_index[1], dtype=np.int64)
    wgt = np.asarray(edge_weight, dtype=np.float32) * (1.0 - cfg.alpha)

    core = dst // cfg.n_loc
    loc = dst - core * cfg.n_loc
    win = loc // 128
    dl = (loc - win * 128).astype(NP_BF16)
    prow = src // cfg.n_loc * cfg.n_loc_pad + src % cfg.n_loc
    sb = prow // cfg.sb_rows
    rel = (prow - sb * cfg.sb_rows).astype(np.int16)

    key = (core * cfg.n_sb + sb) * cfg.n_win + win
    cnt = np.bincount(key, minlength=C * cfg.n_sb * cfg.n_win).reshape(
        C, cfg.n_sb, cfg.n_win
    )
    pad = ((cnt.max(axis=0) + 127) // 128) * 128  # [n_sb, n_win]
    ntile = pad // 128

    # stream layout: [g][sb][w in group] padded buckets
    bucket_off = np.zeros((cfg.n_sb, cfg.n_win), dtype=np.int64)
    chunk_base = np.zeros((cfg.n_grp, cfg.n_sb), dtype=np.int64)
    n_gsb = np.zeros((cfg.n_grp, cfg.n_sb), dtype=np.int64)
    idx_off = np.zeros((cfg.n_grp, cfg.n_sb), dtype=np.int64)
    off = 0
    for g in range(cfg.n_grp):
        for s in range(cfg.n_sb):
            chunk_base[g, s] = off
            idx_off[g, s] = off // 16
            for w in cfg.wins(g):
                bucket_off[s, w] = off
                off += int(pad[s, w])
            n_gsb[g, s] = off - chunk_base[g, s]
    tot_slots = off
    t_tot = tot_slots // 128

    order = np.lexsort((win, sb, core))
    beta = np.log(cfg.theta / (np.arange(cfg.n_layers) + 1) + 1.0).astype(
        np.float32
    )
    weff = np.zeros((H, cfg.n_layers * H), dtype=np.float32)
    for l in range(cfg.n_layers):
        weff[:, l * H : (l + 1) * H] = (1.0 - beta[l]) * np.eye(
            H, dtype=np.float32
        ) + beta[l] * np.asarray(conv_W[l], dtype=np.float32)

    x = np.asarray(x, dtype=np.float32)
    ntmax = int(n_gsb.max()) // 128
    consts = {
        "w0": np.asarray(W0, dtype=np.float32),
        "b0": np.asarray(b0, dtype=np.float32).reshape(H, 1).copy(),
        "w1": np.asarray(W1, dtype=np.float32).astype(NP_BF16),
        "b1": np.asarray(b1, dtype=np.float32).reshape(H, 1).copy(),
        "weff": weff.astype(NP_BF16),
        "iota": np.tile(np.arange(128, dtype=np.float32), (128, 1)).astype(
            NP_BF16
        ),
        "identb": np.eye(128, dtype=np.float32).astype(NP_BF16),
        "identf": np.eye(128, dtype=np.float32),
    }

    in_maps = []
    for c in range(C):
        m = core[order] == c
        o = order[m]
        sb_c, win_c = sb[o], win[o]
        bkey = sb_c * cfg.n_win + win_c
        bc = np.bincount(bkey, minlength=cfg.n_sb * cfg.n_win)
        first = np.zeros(cfg.n_sb * cfg.n_win, dtype=np.int64)
        first[1:] = np.cumsum(bc)[:-1]
        rank = np.arange(bkey.size) - first[bkey]
        slot = bucket_off[sb_c, win_c] + rank

        idx_full = np.zeros(tot_slots, dtype=np.int16)
        dl_full = np.zeros(tot_slots, dtype=NP_BF16)
        w_full = np.zeros(tot_slots, dtype=NP_BF16)
        idx_full[slot] = rel[o]
        dl_full[slot] = dl[o]
        w_full[slot] = wgt[o].astype(NP_BF16)

        # gather index layout: [16, n/16] per region, tiled to 128 partitions
        idx_arr = np.tile(
            idx_full.reshape(tot_slots // 16, 16).T, (8, 1)
        )  # [128, tot_slots/16]

        # host-precomputed scatter matrix S (layer-invariant):
        # S[p, t, d] = (d == dst_rel[p,t]) * w[p,t], stream-tile order.
        dl_arr = dl_full.reshape(t_tot, 128).T
        w_arr = w_full.reshape(t_tot, 128).T
        sc_arr = (
            (
                dl_arr[:, :, None].astype(np.float32)
                == np.arange(128, dtype=np.float32)[None, None, :]
            )
            * w_arr[:, :, None].astype(np.float32)
        ).astype(NP_BF16).reshape(128, t_tot * 128)

        xT = np.zeros((cfg.f_in, cfg.n_loc_pad), dtype=np.float32)
        xT[:, : cfg.n_loc] = x[c * cfg.n_loc : (c + 1) * cfg.n_loc].T

        in_maps.append(dict(consts, xT=xT, idx=idx_arr, sc=sc_arr))

    meta = dict(
        pad=pad,
        ntile=ntile,
        n_gsb=n_gsb,
        bucket_off=bucket_off,
        chunk_base=chunk_base,
        idx_off=idx_off,
        tot_slots=tot_slots,
        t_tot=t_tot,
        ntmax=ntmax,
    )
    return in_maps, meta


# ---------------------------------------------------------------------------
# device program
# ---------------------------------------------------------------------------


def build_program(cfg, meta):
    import contextlib

    H = cfg.h
    NW = cfg.n_win
    NSB = cfg.n_sb
    NLP = cfg.n_loc_pad
    ntile, n_gsb = meta["ntile"], meta["n_gsb"]
    kf = cfg.f_in // 128
    nchunk = (NLP + 511) // 512

    nc = bacc.Bacc("TRN2", target_bir_lowering=False, num_swdge_queues=4)

    xT_d = nc.declare_dram_parameter("xT", [cfg.f_in, NLP], F32, isOutput=False)
    idx_d = nc.declare_dram_parameter(
        "idx", [128, meta["tot_slots"] // 16], I16, isOutput=False
    )
    sc_d = nc.declare_dram_parameter(
        "sc", [128, meta["t_tot"] * 128], BF16, isOutput=False
    )
    w0_d = nc.declare_dram_parameter("w0", [cfg.f_in, H], F32, isOutput=False)
    b0_d = nc.declare_dram_parameter("b0", [H, 1], F32, isOutput=False)
    w1_d = nc.declare_dram_parameter("w1", [H, H], BF16, isOutput=False)
    b1_d = nc.declare_dram_parameter("b1", [H, 1], F32, isOutput=False)
    weff_d = nc.declare_dram_parameter(
        "weff", [H, cfg.n_layers * H], BF16, isOutput=False
    )
    iota_d = nc.declare_dram_parameter("iota", [128, 128], BF16, isOutput=False)
    identb_d = nc.declare_dram_parameter("identb", [128, 128], BF16, isOutput=False)
    identf_d = nc.declare_dram_parameter("identf", [128, 128], F32, isOutput=False)
    out_d = nc.declare_dram_parameter("out", [NLP, H], F32, isOutput=True)

    from concourse import library_config

    with tile.TileContext(nc) as tc, contextlib.ExitStack() as ctx:
        nc.gpsimd.load_library(library_config.mlp)
        dram = ctx.enter_context(tc.tile_pool(name="dram", bufs=1, space="DRAM"))
        consts = ctx.enter_context(tc.tile_pool(name="consts", bufs=1))
        big = ctx.enter_context(tc.tile_pool(name="big", bufs=1))
        xs_pool = ctx.enter_context(tc.tile_pool(name="xs", bufs=3))
        idx_pool = ctx.enter_context(tc.tile_pool(name="idxp", bufs=3))
        dw_pool = ctx.enter_context(tc.tile_pool(name="dwp", bufs=2))
        msg_pools = [
            ctx.enter_context(tc.tile_pool(name=f"msg{s}", bufs=3))
            for s in range(NSB)
        ]
        sch_pools = [
            ctx.enter_context(tc.tile_pool(name=f"sch{s}", bufs=2))
            for s in range(NSB)
        ]
        stage_pool = ctx.enter_context(tc.tile_pool(name="stg", bufs=1))
        ps_agg = ctx.enter_context(tc.tile_pool(name="ps_agg", bufs=4, space="PSUM"))
        ps_big = ctx.enter_context(tc.tile_pool(name="ps_big", bufs=2, space="PSUM"))
        ps_tr = ctx.enter_context(tc.tile_pool(name="ps_tr", bufs=2, space="PSUM"))

        tables = [
            dram.tile(
                [cfg.n_tab, 128], BF16, addr_space="Shared", name=f"table{i}"
            )
            for i in range(cfg.n_layers + 1)
        ]
        staging = dram.tile([NLP, 128], BF16)

        iota_t = consts.tile([128, 128], BF16)
        identb_t = consts.tile([128, 128], BF16)
        identf_t = consts.tile([128, 128], F32)
        w0_t = consts.tile([128, kf * H], F32)
        w1_t = consts.tile([H, H], BF16)
        b0_t = consts.tile([H, 1], F32)
        b1_t = consts.tile([H, 1], F32)
        weff_t = consts.tile([H, cfg.n_layers * H], BF16)

        nc.sync.dma_start(iota_t[:], iota_d[:])
        nc.sync.dma_start(identb_t[:], identb_d[:])
        nc.sync.dma_start(identf_t[:], identf_d[:])
        for k in range(kf):
            nc.sync.dma_start(
                w0_t[:, k * H : (k + 1) * H], w0_d[k * 128 : (k + 1) * 128, :]
            )
        nc.sync.dma_start(w1_t[:], w1_d[:])
        nc.sync.dma_start(b0_t[:], b0_d[:])
        nc.sync.dma_start(b1_t[:], b1_d[:])
        nc.sync.dma_start(weff_t[:], weff_d[:])

        x0s = big.tile([H, NLP], BF16)
        support = big.tile([H, NLP], BF16)
        hT = big.tile([H, NLP], BF16)
        stage_s = stage_pool.tile([128, NW * H], BF16)

        # zero the feature-pad half of the staging buffer, once (reuse
        # stage_s as the zero source; it is rewritten by every layer anyway)
        nc.vector.memset(stage_s[:], 0)
        nc.sync.dma_start(
            staging[:].rearrange("(w p) f -> p w f", p=128)[:, :, H:],
            stage_s[:].rearrange("p (w f) -> p w f", f=H),
        )

        # ---------------- prologue: h0 = relu(x @ W0 + b0) -----------------
        for cix in range(nchunk):
            c0 = cix * 512
            cw = min(512, NLP - c0)
            ps = ps_big.tile([H, 512], F32)
            for k in range(kf):
                xt = xs_pool.tile([128, 512], F32)
                nc.sync.dma_start(
                    xt[:, :cw], xT_d[k * 128 : (k + 1) * 128, c0 : c0 + cw]
                )
                nc.tensor.matmul(
                    ps[:, :cw],
                    lhsT=w0_t[:, k * H : (k + 1) * H],
                    rhs=xt[:, :cw],
                    start=(k == 0),
                    stop=(k == kf - 1),
                )
            nc.scalar.activation(
                hT[:, c0 : c0 + cw], ps[:, :cw], AF.Relu, bias=b0_t[:]
            )
            nc.vector.tensor_scalar_mul(
                x0s[:, c0 : c0 + cw], hT[:, c0 : c0 + cw], cfg.alpha
            )

        def stage_and_allgather(table):
            for w in range(NW):
                pst = ps_tr.tile([128, H], BF16)
                nc.tensor.transpose(
                    out=pst[:],
                    in_=hT[:, w * 128 : (w + 1) * 128],
                    identity=identb_t[:H, :H],
                )
                nc.vector.tensor_copy(
                    out=stage_s[:, w * H : (w + 1) * H], in_=pst[:]
                )
            nc.sync.dma_start(
                staging[:].rearrange("(w p) f -> p w f", p=128)[:, :, :H],
                stage_s[:].rearrange("p (w f) -> p w f", f=H),
            )
            nc.gpsimd.collective_compute(
                "AllGather",
                ALU.bypass,
                ins=[staging[:].opt()],
                outs=[table[:].opt()],
                replica_groups=[list(range(cfg.n_cores))],
            )

        stage_and_allgather(tables[0])

        # ---------------- per-layer body -----------------------------------
        def layer_body(li):
            table = tables[li]
            nc.vector.tensor_copy(out=support[:], in_=x0s[:])
            for g in range(cfg.n_grp):
                tg = int(n_gsb[g].sum()) // 128
                t0g = int(meta["chunk_base"][g, 0]) // 128
                idxn = int(n_gsb[g].sum()) // 16
                if idxn:
                    idx_t = idx_pool.tile([128, idxn], I16)
                    nc.sync.dma_start(
                        idx_t[:],
                        idx_d[:, int(meta["idx_off"][g, 0]) :][:, :idxn],
                    )
                msgs = {}
                schs = {}
                for s in range(NSB):
                    n = int(n_gsb[g, s])
                    if n == 0:
                        continue
                    nt = n // 128
                    mt = msg_pools[s].tile([128, nt, 128], BF16)
                    ic = int(meta["idx_off"][g, s] - meta["idx_off"][g, 0])
                    nc.gpsimd.dma_gather(
                        mt[:],
                        table[s * cfg.sb_rows : (s + 1) * cfg.sb_rows, :],
                        idx_t[:, ic : ic + n // 16],
                        n,
                        n,
                        128,
                        single_packet=False,  # single-packet mode breaks >1k idxs
                        queue_num=s,
                    )
                    msgs[s] = mt
                for s in range(NSB):
                    n = int(n_gsb[g, s])
                    if n == 0:
                        continue
                    nt = n // 128
                    # S chunk for (g, s): host-precomputed (layer-invariant),
                    # streamed sequentially from DRAM — no DVE work at all
                    t0 = int(meta["chunk_base"][g, s]) // 128
                    sch = sch_pools[s].tile([128, nt, 128], BF16)
                    nc.sync.dma_start(
                        sch[:].rearrange("p t d -> p (t d)"),
                        sc_d[:, t0 * 128 : (t0 + nt) * 128],
                    )
                    schs[s] = sch

                for w in cfg.wins(g):
                    kt = int(sum(ntile[s, w] for s in range(NSB)))
                    if kt == 0:
                        continue
                    ps = ps_agg.tile([H, 128], F32)
                    k = 0
                    for s in range(NSB):
                        nt = int(ntile[s, w])
                        if nt == 0:
                            continue
                        i0 = int(
                            meta["bucket_off"][s, w] - meta["chunk_base"][g, s]
                        ) // 128
                        for i in range(nt):
                            nc.tensor.matmul(
                                ps[:],
                                lhsT=msgs[s][:, i0 + i, :H],
                                rhs=schs[s][:, i0 + i, :],
                                start=(k == 0),
                                stop=(k == kt - 1),
                            )
                            k += 1
                    nc.vector.tensor_tensor(
                        out=support[:, w * 128 : (w + 1) * 128],
                        in0=support[:, w * 128 : (w + 1) * 128],
                        in1=ps[:],
                        op=ALU.add,
                    )

            if isinstance(li, int):
                wsl = weff_t[:, li * H : (li + 1) * H]
            else:
                # walrus can't take a register offset in ldweights: copy the
                # dynamic Weff slice to a fixed location first
                wsl_t = consts.tile([H, H], BF16, tag="wsl")
                nc.vector.tensor_copy(
                    out=wsl_t[:], in_=weff_t[:, bass.ds(li * H, H)]
                )
                wsl = wsl_t[:]
            for cix in range(nchunk):
                c0 = cix * 512
                cw = min(512, NLP - c0)
                ps = ps_big.tile([H, 512], F32)
                nc.tensor.matmul(
                    ps[:, :cw],
                    lhsT=wsl,
                    rhs=support[:, c0 : c0 + cw],
                    start=True,
                    stop=True,
                )
                nc.scalar.activation(hT[:, c0 : c0 + cw], ps[:, :cw], AF.Relu)
            stage_and_allgather(tables[li + 1])

        if cfg.use_for_i:
            with tc.For_i(0, cfg.n_layers, 1) as li:
                layer_body(li)
        else:
            for li in range(cfg.n_layers):
                layer_body(li)

        # ---------------- epilogue: logits + log_softmax, chunk-wise --------
        out_view = out_d[:].rearrange("(w p) f -> p w f", p=128)
        for cix in range(nchunk):
            c0 = cix * 512
            cw = min(512, NLP - c0)
            nwc = cw // 128  # windows in this chunk
            ps = ps_big.tile([H, 512], F32)
            nc.tensor.matmul(
                ps[:, :cw], lhsT=w1_t[:], rhs=hT[:, c0 : c0 + cw],
                start=True, stop=True,
            )
            lt = xs_pool.tile([H, 512], F32, tag="lt")
            nc.scalar.activation(
                lt[:, :cw], ps[:, :cw], AF.Identity, bias=b1_t[:]
            )
            xch = xs_pool.tile([128, nwc * H], F32, tag="xch")
            for wi in range(nwc):
                pst = ps_tr.tile([128, H], F32)
                nc.tensor.transpose(
                    out=pst[:],
                    in_=lt[:, wi * 128 : (wi + 1) * 128],
                    identity=identf_t[:H, :H],
                )
                nc.vector.tensor_copy(
                    out=xch[:, wi * H : (wi + 1) * H], in_=pst[:]
                )
            xch3 = xch[:].rearrange("p (w f) -> p w f", f=H)
            mx = xs_pool.tile([128, nwc], F32, tag="mx")
            nc.vector.tensor_reduce(
                out=mx[:], in_=xch3, axis=mybir.AxisListType.X, op=ALU.max
            )
            nc.vector.tensor_tensor(
                out=xch3,
                in0=xch3,
                in1=mx[:].unsqueeze(2).to_broadcast([128, nwc, H]),
                op=ALU.subtract,
            )
            enm = xs_pool.tile([128, nwc * H], F32, tag="enm")
            nc.scalar.activation(enm[:], xch[:], AF.Exp)
            ssum = xs_pool.tile([128, nwc], F32, tag="ssum")
            nc.vector.tensor_reduce(
                out=ssum[:],
                in_=enm[:].rearrange("p (w f) -> p w f", f=H),
                axis=mybir.AxisListType.X,
                op=ALU.add,
            )
            lsum = xs_pool.tile([128, nwc], F32, tag="lsum")
            nc.scalar.activation(lsum[:], ssum[:], AF.Ln)
            nc.vector.tensor_tensor(
                out=xch3,
                in0=xch3,
                in1=lsum[:].unsqueeze(2).to_broadcast([128, nwc, H]),
                op=ALU.subtract,
            )
            nc.sync.dma_start(
                out_view[:, cix * 4 : cix * 4 + nwc, :], xch3
            )

    nc.compile()
    return nc


# ---------------------------------------------------------------------------
# entry point
# ---------------------------------------------------------------------------


LAST_EXEC_NS = None
LAST_TRACE = None


def kernel(x, edge_index, edge_weight, W0, b0, W1, b1, conv_W):
    global LAST_EXEC_NS, LAST_TRACE
    from concourse.bass_utils import run_bass_kernel_spmd

    cfg = Cfg()
    in_maps, meta = preprocess(
        cfg, x, edge_index, edge_weight, W0, b0, W1, b1, conv_W
    )
    nc = build_program(cfg, meta)
    res = run_bass_kernel_spmd(nc, in_maps, list(range(cfg.n_cores)))
    LAST_EXEC_NS = getattr(res, "exec_time_ns", None)
    LAST_TRACE = getattr(res, "instructions_and_trace", None)
    outs = res.results
    full = np.concatenate(
        [np.asarray(outs[c]["out"])[: cfg.n_loc] for c in range(cfg.n_cores)],
        axis=0,
    )
    return full.astype(np.float32)

